# revision 1
# baseline (speedup 1.0000x reference)
"""DiffPoolEncoder Trainium2 kernel.

Sharding: data parallel by graph. 8 cores x 4 graphs (512 nodes each).
Per core the whole network runs on-device; GraphSage aggregation uses dense
per-graph A^T tiles built on-device via gpsimd local_scatter from
host-prepared (dst, count) index tables (index-only preprocessing of the
edge list). Heavy matmuls run in float32r (TF32, 1 cycle/row); adjacency
counts and 1/deg (deg = 2^k) are exactly representable, so the aggregation
matrix itself is exact. Activations keep dual layout: feature-major from
the linears, node-major via PE transposes. SBUF pools are strictly
LIFO-nested by lifetime epoch.
"""

import sys

for _p in ("/opt/trn_rl_repo",):
    if _p not in sys.path:
        sys.path.append(_p)

import numpy as np
import ml_dtypes
from contextlib import ExitStack

import concourse.bass as bass
import concourse.mybir as mybir
import concourse.tile as tile
from concourse import bacc
from concourse.bass_utils import run_bass_kernel_spmd

F32 = mybir.dt.float32
F32R = mybir.dt.float32r
BF16 = mybir.dt.bfloat16
I16 = mybir.dt.int16
AF = mybir.ActivationFunctionType
ALU = mybir.AluOpType
AX = mybir.AxisListType

NCORES = 8
B = 32
NPG = 512
G = 4            # graphs per core
T = 16           # node tiles per core (4 per graph)
NLOC = 2048      # nodes per core
K = 64           # clusters per graph
IN = 128
HID = 256
NI = 48          # padded (dst,count) entries per (src-tile, partition)

# bcol column layout (each 128-chunk of a bias vector is one column)
BC_B1, BC_B2, BC_B3 = 0, 2, 4
BC_AB1, BC_AB2 = 6, 8
BC_AB3 = 10          # 16 cols
BC_QB1, BC_QB2, BC_QB3 = 26, 28, 30
BC_MB1, BC_MB2 = 32, 34
BC_N = 35

# rows2 [65, 1536] f32r: rows at matmul base partitions {0, 32, 64};
# ones[0:512] replicated at each used partition (matmul needs equal bases).
R_QB1 = (0, 512)
R_QB2, R_QB3 = (64, 512), (64, 768)
R_PB = (32, 512)     # 256 (per-core pW bias slice)
ROWS_W = 1024


def build_module():
    nc = bacc.Bacc("TRN2", target_bir_lowering=False)

    # ---------------- DRAM I/O ----------------
    featT_d = nc.dram_tensor("featT", [128, NLOC], F32R, kind="ExternalInput")
    featnm_d = nc.dram_tensor("feat_nm", [128, T * IN], F32R, kind="ExternalInput")
    atidx_d = nc.dram_tensor("at_idx", [128, T * NI], I16, kind="ExternalInput")
    atval_d = nc.dram_tensor("at_val", [128, T * NI], BF16, kind="ExternalInput")
    bcol_d = nc.dram_tensor("bcol", [128, BC_N], F32, kind="ExternalInput")
    rows_d = nc.dram_tensor("rows2", [65, ROWS_W], F32R, kind="ExternalInput")
    ident_d = nc.dram_tensor("ident", [128, 128], F32, kind="ExternalInput")
    identr_d = nc.dram_tensor("identr", [128, 128], F32R, kind="ExternalInput")
    w_d = {}
    for name, fi, fo in [
        ("W1", 256, 256), ("W2", 512, 256), ("W3", 512, 256),
        ("aW1", 256, 256), ("aW2", 512, 256), ("aW3", 512, 2048),
        ("pWl", 2560, 256), ("qW1", 1536, 256), ("qW2", 512, 256),
        ("qW3", 512, 256), ("mW1", 1536, 256), ("mW2", 256, 10),
    ]:
        w_d[name] = nc.dram_tensor(name, [fi, fo], F32R, kind="ExternalInput")
    yp_d = nc.dram_tensor("yp", [10, G], F32, kind="ExternalOutput")

    with tile.TileContext(nc) as tc, ExitStack() as ex, \
            nc.allow_low_precision(reason="f32r is tf32; accumulation stays fp32 in PSUM"):
        persist = ex.enter_context(tc.tile_pool(name="persist", bufs=1))
        # PSUM: 8 banks. One tag per pool so slot count == bank count.
        ps_p = ex.enter_context(tc.tile_pool(name="psP", bufs=4, space="PSUM"))
        lg_p = ex.enter_context(tc.tile_pool(name="psL", bufs=1, space="PSUM"))
        pm_p = ex.enter_context(tc.tile_pool(name="psM", bufs=2, space="PSUM"))
        pl_p = ex.enter_context(tc.tile_pool(name="psS", bufs=1, space="PSUM"))
        dram = ex.enter_context(tc.tile_pool(name="dram", bufs=1, space="DRAM"))

        uid = [0]

        def _nm(pfx):
            uid[0] += 1
            return f"{pfx}{uid[0]}"

        def ps_big(dt=F32):
            return ps_p.tile([128, 512], dt, tag="ps", name=_nm("ps"))

        def ps_med(p, f, dt=F32):
            return pm_p.tile([p, f], dt, tag="pm", name=_nm("pm"))

        def ps_sml(p, f, dt=F32):
            return pl_p.tile([p, f], dt, tag="pl", name=_nm("pl"))

        def wload(pool, name, fi, fo):
            kk = fi // 128
            sb = pool.tile([128, kk * fo], F32R, tag=name, name=name)
            nc.sync.dma_start(
                sb[:].rearrange("p (k f) -> p k f", k=kk, f=fo),
                w_d[name][:, :].rearrange("(k p) f -> p k f", p=128),
            )
            return sb

        # ---------- persistent small tensors (epoch E6) ----------
        ident = persist.tile([128, 128], F32)
        identr = persist.tile([128, 128], F32R)
        rows2 = persist.tile([65, ROWS_W], F32R)
        bcol = persist.tile([128, BC_N], F32)
        ones_c = persist.tile([128, 1], F32R)
        degcl = persist.tile([128, T], F32)      # clamped deg, node-major cols
        S_nm = persist.tile([128, T * K], F32R)
        out_fm = persist.tile([128, 12 * G], F32R)  # readout maxes, col=ch*G+g
        nmax = persist.tile([128, 2], F32)
        sumx = persist.tile([128, 2], F32)
        y_sb = persist.tile([128, 2 * G], F32R)
        z_sb = persist.tile([10, G], F32)
        nc.sync.dma_start(ident[:], ident_d[:])
        nc.sync.dma_start(identr[:], identr_d[:])
        nc.sync.dma_start(rows2[:], rows_d[:])
        nc.sync.dma_start(bcol[:], bcol_d[:])
        nc.vector.memset(ones_c[:].bitcast(F32), 1.0)

        def ones_at(p, n):
            return rows2[p : p + 1, 0:n]

        def rrow(ro, n):
            p, off = ro
            return rows2[p : p + 1, off : off + n]

        # AT (scaled A^T tiles, f32) lives to the end (epoch E6)
        at_p = ex.enter_context(tc.tile_pool(name="atp", bufs=1))
        AT = at_p.tile([128, T * NPG], F32R)

        # ---------- LIFO phase pools ----------
        ex5 = ExitStack()   # close after logits/softmax
        agg_p = ex5.enter_context(tc.tile_pool(name="aggfm", bufs=2))
        afm2_p = ex5.enter_context(tc.tile_pool(name="afm2", bufs=1))
        ex3 = ExitStack()   # close after h3 (h3 runs after xnm closes)
        xfm_p = ex3.enter_context(tc.tile_pool(name="xfm", bufs=2))
        w3_p = ex3.enter_context(tc.tile_pool(name="w3p", bufs=1))
        ex4 = ExitStack()   # close after agg_a2
        xnm_p = ex4.enter_context(tc.tile_pool(name="xnm", bufs=2))
        ex2c = ExitStack()  # close after a2
        a2w_p = ex2c.enter_context(tc.tile_pool(name="a2wp", bufs=1))
        afm1_p = ex2c.enter_context(tc.tile_pool(name="afm1", bufs=1))
        ex2b = ExitStack()  # close after h2
        w2_p = ex2b.enter_context(tc.tile_pool(name="w2p", bufs=1))
        ex2 = ExitStack()   # close after h1/a1
        w1_p = ex2.enter_context(tc.tile_pool(name="w1p", bufs=1))
        ex1 = ExitStack()   # close after agg_feat
        fnm_p = ex1.enter_context(tc.tile_pool(name="fnmp", bufs=1))
        ex0 = ExitStack()   # close after A^T built+scaled
        ate_p = ex0.enter_context(tc.tile_pool(name="atep", bufs=1))

        # ---------- input DMAs ----------
        atbf_idx = ate_p.tile([128, T * NI], I16, tag="atidx")
        atbf_val = ate_p.tile([128, T * NI], BF16, tag="atval")
        nc.sync.dma_start(atbf_idx[:], atidx_d[:])
        nc.sync.dma_start(atbf_val[:], atval_d[:])
        featnm = fnm_p.tile([128, T * IN], F32R, tag="featnm")
        nc.sync.dma_start(featnm[:], featnm_d[:])
        W1 = wload(w1_p, "W1", 256, 256)
        aW1 = wload(w1_p, "aW1", 256, 256)
        W2 = wload(w2_p, "W2", 512, 256)
        W3 = wload(w3_p, "W3", 512, 256)
        aW2 = wload(a2w_p, "aW2", 512, 256)

        # ---------- phase 0: build scaled A^T ----------
        for t in range(T):
            scr = ate_p.tile([128, NPG], BF16, tag="scscr", name=_nm("sc"), bufs=1)
            nc.gpsimd.local_scatter(
                out_ap=scr[:],
                data_ap=atbf_val[:, t * NI : (t + 1) * NI],
                idxs_ap=atbf_idx[:, t * NI : (t + 1) * NI],
                channels=128, num_elems=NPG, num_idxs=NI,
            )
            nc.vector.tensor_copy(AT[:, t * NPG : (t + 1) * NPG], scr[:])

        # deg rows -> dinv rows (partitions 0/32 of a [33,1024] tile)
        dinvsb = ate_p.tile([33, 1024], F32R, tag="dinvsb")
        gslot = [(0, 0), (0, 512), (32, 0), (32, 512)]
        for g in range(G):
            p, off = gslot[g]
            dps = ps_big()
            for st in range(4):
                nc.tensor.matmul(dps[p : p + 1, :], lhsT=ones_c[:].bitcast(F32),
                                 rhs=AT[:, (g * 4 + st) * NPG : (g * 4 + st + 1) * NPG].bitcast(F32),
                                 start=(st == 0), stop=(st == 3),
                                 skip_group_check=True)
            nc.vector.tensor_scalar(dinvsb[p : p + 1, off : off + NPG],
                                    dps[p : p + 1, :], 1.0, None, op0=ALU.max)
            nc.vector.reciprocal(dinvsb[p : p + 1, off : off + NPG],
                                 dinvsb[p : p + 1, off : off + NPG])
        # clamped deg as node-major columns (from raw A^T)
        for t in range(T):
            g, j = t // 4, t % 4
            cps = ps_sml(128, 1)
            for st in range(4):
                nc.tensor.matmul(
                    cps[:],
                    lhsT=AT[:, (g * 4 + st) * NPG + j * 128 : (g * 4 + st) * NPG + (j + 1) * 128].bitcast(F32),
                    rhs=ones_c[:].bitcast(F32), start=(st == 0), stop=(st == 3))
            nc.vector.tensor_scalar(degcl[:, t : t + 1], cps[:], 1.0, None, op0=ALU.max)
        # AT <- AT * dinv[dst] (broadcast via K=1 outer product, per graph)
        for g in range(G):
            p, off = gslot[g]
            bps = ps_big()
            nc.tensor.matmul(bps[:], lhsT=ones_at(p, 128).bitcast(F32),
                             rhs=dinvsb[p : p + 1, off : off + NPG].bitcast(F32),
                             start=True, stop=True)
            dbc = ate_p.tile([128, NPG], F32R, tag="dbc", name=_nm("dbc"), bufs=1)
            nc.scalar.copy(dbc[:], bps[:])
            for st in range(4):
                t = g * 4 + st
                nc.vector.tensor_tensor(
                    out=AT[:, t * NPG : (t + 1) * NPG],
                    in0=AT[:, t * NPG : (t + 1) * NPG],
                    in1=dbc[:], op=ALU.mult)
        ex0.close()
        ft_p_stack = ExitStack()   # close after h1/a1
        ft_p = ft_p_stack.enter_context(tc.tile_pool(name="ftp", bufs=1))
        featT = ft_p.tile([128, NLOC], F32R, tag="featT")
        nc.sync.dma_start(featT[:], featT_d[:])

        # ---------- emit helpers ----------
        def emit_agg(x_nm, D, out_t):
            """out_t[d, n] (feature-major) = sum_s x_nm[s, d] * AT[s, n]."""
            for g in range(G):
                for ch in range(D // 128):
                    ps = ps_big()
                    for st in range(4):
                        t = g * 4 + st
                        nc.tensor.matmul(
                            ps[:],
                            lhsT=x_nm[:, t * D + ch * 128 : t * D + ch * 128 + 128],
                            rhs=AT[:, t * NPG : (t + 1) * NPG],
                            start=(st == 0), stop=(st == 3))
                    nc.scalar.copy(
                        out_t[:, ch * NLOC + g * NPG : ch * NLOC + (g + 1) * NPG],
                        ps[:])

        def emit_lin_fm(x_fm, a_fm, Din, Dout, Wsb, bccol, relu, out_t):
            nk = Din // 128
            for co in range(Dout // 128):
                for nb in range(4):
                    ps = ps_big()
                    ki = 0
                    for src in (x_fm, a_fm):
                        for ci in range(nk):
                            nc.tensor.matmul(
                                ps[:],
                                lhsT=Wsb[:, ki * Dout + co * 128 : ki * Dout + co * 128 + 128],
                                rhs=src[:, ci * NLOC + nb * 512 : ci * NLOC + (nb + 1) * 512],
                                start=(ki == 0), stop=(ki == 2 * nk - 1))
                            ki += 1
                    nc.scalar.activation(
                        out_t[:, co * NLOC + nb * 512 : co * NLOC + (nb + 1) * 512],
                        ps[:], AF.Relu if relu else AF.Identity,
                        bias=bcol[:, bccol + co : bccol + co + 1])

        def emit_nm_T(x_fm, out_nm, act=False, spill=None):
            # node-major via PE transposes of the (already relu'd) fm tensor;
            # 4 transposed blocks share one PSUM bank -> single 512-wide evac.
            for t2 in range(0, T, 2):
                tp = ps_big(F32R)
                for i, (t, ch) in enumerate(
                        ((t2, 0), (t2, 1), (t2 + 1, 0), (t2 + 1, 1))):
                    nc.tensor.matmul(
                        tp[:, i * 128 : (i + 1) * 128],
                        lhsT=x_fm[:, ch * NLOC + t * 128 : ch * NLOC + (t + 1) * 128],
                        rhs=identr[:], is_transpose=True,
                        start=True, stop=True, skip_group_check=True)
                if spill is not None:
                    buf, dst_dram = spill
                    sb = buf()
                    nc.vector.tensor_copy(sb[:], tp[:])
                    nc.sync.dma_start(
                        dst_dram[:, t2 * HID : (t2 + 2) * HID], sb[:])
                else:
                    dst = out_nm[:, t2 * HID : (t2 + 2) * HID]
                    if act:
                        nc.scalar.copy(dst, tp[:])
                    else:
                        nc.vector.tensor_copy(dst, tp[:])

        def emit_out1(x_fm, ch0):
            for ci in range(2):
                for g in range(G):
                    nc.vector.tensor_reduce(
                        out_fm[:, (ch0 + ci) * G + g : (ch0 + ci) * G + g + 1],
                        x_fm[:, ci * NLOC + g * NPG : ci * NLOC + (g + 1) * NPG],
                        axis=AX.X, op=ALU.max)

        # ---------- GC stacks ----------
        aggfeat = agg_p.tile([128, NLOC], F32R, tag="agg", name="aggfeat")
        emit_agg(featnm, IN, aggfeat)

        h1f = xfm_p.tile([128, 2 * NLOC], F32R, tag="xfm", name="h1f")
        h1n = xnm_p.tile([128, T * HID], F32R, tag="xnm", name="h1n")
        emit_lin_fm(featT, aggfeat, 128, 256, W1, BC_B1, True, h1f)
        emit_nm_T(h1f, h1n)
        h1spill = dram.tile([128, T * HID], F32R, tag="h1d", name="h1d")
        nc.sync.dma_start(h1spill[:], h1n[:])
        emit_out1(h1f, 0)

        a1f = afm1_p.tile([128, 2 * NLOC], F32R, tag="a1f", name="a1f")
        a1n = xnm_p.tile([128, T * HID], F32R, tag="xnm", name="a1n")
        emit_lin_fm(featT, aggfeat, 128, 256, aW1, BC_AB1, True, a1f)
        a1spill = dram.tile([128, 2 * NLOC], F32R, tag="a1d", name="a1d")
        nc.sync.dma_start(a1spill[:], a1f[:])
        emit_nm_T(a1f, a1n, act=True)
        ft_p_stack.close()
        ex1.close()
        ex2.close()

        aggh1 = agg_p.tile([128, 2 * NLOC], F32R, tag="agg", name="aggh1")
        emit_agg(h1n, HID, aggh1)

        agga1 = agg_p.tile([128, 2 * NLOC], F32R, tag="agg", name="agga1")
        emit_agg(a1n, HID, agga1)

        h2f = xfm_p.tile([128, 2 * NLOC], F32R, tag="xfm", name="h2f")
        h2n = xnm_p.tile([128, T * HID], F32R, tag="xnm", name="h2n")
        emit_lin_fm(h1f, aggh1, 256, 256, W2, BC_B2, True, h2f)
        emit_nm_T(h2f, h2n)
        h2spill = dram.tile([128, T * HID], F32R, tag="h2d", name="h2d")
        nc.sync.dma_start(h2spill[:], h2n[:])
        emit_out1(h2f, 2)
        ex2b.close()

        a2f = afm2_p.tile([128, 2 * NLOC], F32R, tag="a2f", name="a2f")
        a2n = xnm_p.tile([128, T * HID], F32R, tag="xnm", name="a2n")
        emit_lin_fm(a1f, agga1, 256, 256, aW2, BC_AB2, True, a2f)
        emit_nm_T(a2f, a2n, act=True)
        ex2c.close()

        aggh2 = agg_p.tile([128, 2 * NLOC], F32R, tag="agg", name="aggh2")
        emit_agg(h2n, HID, aggh2)

        agga2 = agg_p.tile([128, 2 * NLOC], F32R, tag="agg", name="agga2")
        emit_agg(a2n, HID, agga2)
        ex4.close()

        # late weights issue here so their DMA overlaps h3 compute
        ex5b = ExitStack()
        wl_p = ex5b.enter_context(tc.tile_pool(name="wlate", bufs=1))
        aW3 = wl_p.tile([128, 4 * 2048], F32R, tag="aW3", name="aW3")
        aW3v = aW3[:].rearrange("p (k f) -> p k f", k=4, f=2048)
        for q in range(4):
            nc.sync.dma_start(
                aW3v[:, :, q * 512 : (q + 1) * 512],
                w_d["aW3"][:, q * 512 : (q + 1) * 512].rearrange(
                    "(k p) f -> p k f", p=128))
        pWl = wload(wl_p, "pWl", 2560, 256)
        a1r = wl_p.tile([128, 2 * NLOC], F32R, tag="a1r", name="a1r")
        nc.sync.dma_start(a1r[:], a1spill[:])
        lgs_nm = wl_p.tile([128, T * K], F32, tag="lgs", name="lgs_nm")

        # h3: fm + readout; node-major streamed straight to DRAM
        h3f = xfm_p.tile([128, 2 * NLOC], F32R, tag="xfm", name="h3f")
        h3spill = dram.tile([128, T * HID], F32R, tag="h3d", name="h3d")
        emit_lin_fm(h2f, aggh2, 256, 256, W3, BC_B3, False, h3f)

        def h3buf():
            return w3_p.tile([128, 512], F32R, tag="h3buf", name=_nm("h3b"), bufs=2)

        emit_nm_T(h3f, None, spill=(h3buf, h3spill))
        emit_out1(h3f, 4)

        # ---------- a3 + logits (streamed per graph) ----------
        for g in range(G):
            lps = lg_p.tile([64, 512], F32, tag="lg", name=_nm("lg"))
            for co in range(16):  # a3 = relu(cat(a2, agg_a2) @ aW3 + ab3)
                ps3 = ps_big()
                ki = 0
                for src in (a2f, agga2):
                    for ci in range(2):
                        nc.tensor.matmul(
                            ps3[:],
                            lhsT=aW3[:, ki * 2048 + co * 128 : ki * 2048 + co * 128 + 128],
                            rhs=src[:, ci * NLOC + g * NPG : ci * NLOC + (g + 1) * NPG],
                            start=(ki == 0), stop=(ki == 3))
                        ki += 1
                a3b = wl_p.tile([128, 512], F32R, tag="a3buf", name=_nm("a3b"), bufs=2)
                nc.scalar.activation(a3b[:], ps3[:], AF.Relu,
                                     bias=bcol[:, BC_AB3 + co : BC_AB3 + co + 1])
                nc.tensor.matmul(
                    lps[:], lhsT=pWl[:, (4 + co) * 256 + g * K : (4 + co) * 256 + g * K + K],
                    rhs=a3b[:], start=(co == 0), stop=False)
            for ci in range(2):  # a1 block of pW
                nc.tensor.matmul(
                    lps[:], lhsT=pWl[:, ci * 256 + g * K : ci * 256 + g * K + K],
                    rhs=a1r[:, ci * NLOC + g * NPG : ci * NLOC + (g + 1) * NPG],
                    start=False, stop=False)
            for ci in range(2):  # a2 block
                nc.tensor.matmul(
                    lps[:], lhsT=pWl[:, (2 + ci) * 256 + g * K : (2 + ci) * 256 + g * K + K],
                    rhs=a2f[:, ci * NLOC + g * NPG : ci * NLOC + (g + 1) * NPG],
                    start=False, stop=False)
            nc.tensor.matmul(lps[:],
                             lhsT=rows2[32:33, R_PB[1] + g * K : R_PB[1] + (g + 1) * K].bitcast(F32),
                             rhs=ones_at(32, 512).bitcast(F32), start=False, stop=True)
            lgf = wl_p.tile([64, 512], F32, tag="lgf", name=_nm("lgf"), bufs=1)
            nc.scalar.copy(lgf[:], lps[:])
            for j in range(4):  # transpose to node-major
                t = g * 4 + j
                tps = ps_med(128, 64)
                nc.tensor.transpose(tps[:], lgf[0:64, j * 128 : (j + 1) * 128],
                                    ident[0:64, 0:64])
                nc.vector.tensor_copy(lgs_nm[:, t * K : (t + 1) * K], tps[:])

        # masked softmax == per-graph softmax over K columns
        for t in range(T):
            bb = t % 2
            nc.vector.tensor_reduce(nmax[:, bb : bb + 1], lgs_nm[:, t * K : (t + 1) * K],
                                    axis=AX.X, op=ALU.max, negate=True)
            nc.scalar.activation(S_nm[:, t * K : (t + 1) * K],
                                 lgs_nm[:, t * K : (t + 1) * K], AF.Exp,
                                 bias=nmax[:, bb : bb + 1],
                                 accum_out=sumx[:, bb : bb + 1])
            nc.vector.reciprocal(sumx[:, bb : bb + 1], sumx[:, bb : bb + 1])
            nc.vector.tensor_scalar(S_nm[:, t * K : (t + 1) * K],
                                    S_nm[:, t * K : (t + 1) * K],
                                    sumx[:, bb : bb + 1], None, op0=ALU.mult)
        ex5b.close()
        ex3.close()
        ex5.close()

        # ---------- late pool: pooled stage ----------
        late = ex.enter_context(tc.tile_pool(name="late", bufs=1))
        h1r = late.tile([128, T * HID], F32R, tag="h1r", name="h1r")
        h2r = late.tile([128, T * HID], F32R, tag="h2r", name="h2r")
        h3r = late.tile([128, T * HID], F32R, tag="h3r", name="h3r")
        for q in range(4):
            sl = slice(q * 4 * HID, (q + 1) * 4 * HID)
            nc.sync.dma_start(h1r[:, sl], h1spill[:, sl])
            nc.sync.dma_start(h2r[:, sl], h2spill[:, sl])
            nc.sync.dma_start(h3r[:, sl], h3spill[:, sl])
        Xr = [h1r, h2r, h3r]
        qW1 = wload(late, "qW1", 1536, 256)
        qW2 = wload(late, "qW2", 512, 256)
        qW3 = wload(late, "qW3", 512, 256)
        mW1 = wload(late, "mW1", 1536, 256)
        mW2 = wload(late, "mW2", 256, 10)
        AS_nm = late.tile([128, T * K], F32R, tag="AS", name="AS_nm")

        # AS = A @ S: scaled-AT product un-scaled by clamped deg (exact)
        for t in range(T):
            g, j = t // 4, t % 4
            ps = ps_sml(128, K)
            for st in range(4):
                nc.tensor.matmul(
                    ps[:],
                    lhsT=AT[:, (g * 4 + st) * NPG + j * 128 : (g * 4 + st) * NPG + (j + 1) * 128],
                    rhs=S_nm[:, (g * 4 + st) * K : (g * 4 + st + 1) * K],
                    start=(st == 0), stop=(st == 3))
            nc.vector.tensor_scalar(AS_nm[:, t * K : (t + 1) * K], ps[:],
                                    degcl[:, t : t + 1], None, op0=ALU.mult)

        # ---------- h_pool = S^T X, pair-stacked [128 = 2 graphs, .] ----------
        hp_nm = late.tile([128, 2 * 768], F32R, tag="hpn", name="hp_nm")
        hp_fm = late.tile([128, 6 * 256], F32R, tag="hpf", name="hp_fm")
        for h in range(2):
            for L in range(3):
                for gs in range(2):
                    g = h * 2 + gs
                    ps = ps_med(64, 256)
                    for j in range(4):
                        t = g * 4 + j
                        nc.tensor.matmul(
                            ps[:],
                            lhsT=S_nm[:, t * K : (t + 1) * K],
                            rhs=Xr[L][:, t * HID : (t + 1) * HID],
                            start=(j == 0), stop=(j == 3))
                    dst = hp_nm[gs * 64 : gs * 64 + 64,
                                h * 768 + L * 256 : h * 768 + (L + 1) * 256]
                    if gs == 0:
                        nc.vector.tensor_copy(dst, ps[:])
                    else:
                        sh = late.tile([64, 256], F32R, tag="hpsh",
                                       name=_nm("hpsh"), bufs=2)
                        nc.vector.tensor_copy(sh[:], ps[:])
                        nc.sync.dma_start(dst, sh[:])
            for ch in range(6):  # hp_fm via transposes of the pair tile
                tp = ps_med(128, 128, F32R)
                nc.tensor.transpose(
                    tp[:], hp_nm[:, h * 768 + ch * 128 : h * 768 + (ch + 1) * 128],
                    identr[:])
                nc.vector.tensor_copy(
                    hp_fm[:, ch * 256 + h * 128 : ch * 256 + (h + 1) * 128], tp[:])

        # ---------- adj = S^T (A S), pair-stacked; row-normalized ----------
        adjg = late.tile([128, 2 * K], F32, tag="adjg", name="adjg")
        rsum = late.tile([128, 2], F32, tag="rsum", name="rsum")
        adjT = late.tile([128, 2 * 128], F32R, tag="adjT", name="adjT")
        nc.vector.memset(adjT[:].bitcast(F32), 0.0)
        for h in range(2):
            for gs in range(2):
                g = h * 2 + gs
                ps = ps_sml(64, K)
                for j in range(4):
                    t = g * 4 + j
                    nc.tensor.matmul(ps[:],
                                     lhsT=S_nm[:, t * K : (t + 1) * K],
                                     rhs=AS_nm[:, t * K : (t + 1) * K],
                                     start=(j == 0), stop=(j == 3))
                dst = adjg[gs * 64 : gs * 64 + 64, h * K : (h + 1) * K]
                if gs == 0:
                    nc.vector.tensor_copy(dst, ps[:])
                else:
                    sh = late.tile([64, K], F32, tag="adsh",
                                   name=_nm("adsh"), bufs=2)
                    nc.vector.tensor_copy(sh[:], ps[:])
                    nc.sync.dma_start(dst, sh[:])
            nc.vector.tensor_reduce(rsum[:, h : h + 1], adjg[:, h * K : (h + 1) * K],
                                    axis=AX.X, op=ALU.add)
            nc.vector.tensor_scalar(rsum[:, h : h + 1], rsum[:, h : h + 1],
                                    1e-9, None, op0=ALU.add)
            nc.vector.reciprocal(rsum[:, h : h + 1], rsum[:, h : h + 1])
            nc.vector.tensor_scalar(adjg[:, h * K : (h + 1) * K],
                                    adjg[:, h * K : (h + 1) * K],
                                    rsum[:, h : h + 1], None, op0=ALU.mult)
            # transpose each graph's [64,64] block onto the block diagonal
            # (transpose outputs must land at PSUM partition 0; odd block is
            # partition-shifted into place with a small SBUF->SBUF DMA)
            for gs in range(2):
                tp = ps_sml(128, K)
                nc.tensor.transpose(
                    tp[0:64, :],
                    adjg[gs * 64 : gs * 64 + 64, h * K : (h + 1) * K],
                    ident[gs * 64 : gs * 64 + 64, gs * 64 : gs * 64 + 64]
                    if gs else ident[0:64, 0:64])
                if gs == 0:
                    nc.vector.tensor_copy(adjT[0:64, h * 128 : h * 128 + 64],
                                          tp[0:64, :])
                else:
                    sb = late.tile([64, K], F32R, tag="adjsh", name=_nm("adjsh"),
                                   bufs=2)
                    nc.vector.tensor_copy(sb[:], tp[0:64, :])
                    nc.sync.dma_start(
                        adjT[64:128, h * 128 + 64 : h * 128 + 128], sb[:])

        # ---------- pooled sage stack (pair-batched) ----------
        hn1_fm = late.tile([128, 6 * 256], F32R, tag="hn1", name="hn1_fm")
        p1_nm = late.tile([128, 2 * 256], F32R, tag="p1n", name="p1_nm")
        p1_fm = late.tile([128, 2 * 256], F32R, tag="p1f", name="p1_fm")
        hn2_fm = late.tile([128, 2 * 256], F32R, tag="hn2", name="hn2_fm")
        p2_nm = late.tile([128, 2 * 256], F32R, tag="p2n", name="p2_nm")
        p2_fm = late.tile([128, 2 * 256], F32R, tag="p2f", name="p2_fm")
        hn3_fm = late.tile([128, 2 * 256], F32R, tag="hn3", name="hn3_fm")
        p3_fm = late.tile([128, 2 * 256], F32R, tag="p3f", name="p3_fm")

        def pool_hn(x_nm, xw, out_t):
            # out[d, u-pair] = sum_{v-pair} x_nm[v, d] * adjT_bd[v, u]
            for h in range(2):
                for ch in range(xw // 128):
                    tp = ps_sml(128, 128)
                    nc.tensor.matmul(
                        tp[:],
                        lhsT=x_nm[:, h * xw + ch * 128 : h * xw + (ch + 1) * 128],
                        rhs=adjT[:, h * 128 : (h + 1) * 128],
                        start=True, stop=True)
                    nc.vector.tensor_copy(
                        out_t[:, ch * 256 + h * 128 : ch * 256 + (h + 1) * 128], tp[:])

        def pool_lin(xf, hf, Din, Wsb, bccol, rbias, relu, outf, outn):
            nch = Din // 256
            for co in range(2):
                ps = ps_med(128, 256)
                ki = 0
                for src in (xf, hf):
                    for ch in range(nch):
                        nc.tensor.matmul(
                            ps[:],
                            lhsT=Wsb[:, ki * 256 + co * 128 : ki * 256 + co * 128 + 128],
                            rhs=src[:, ch * 256 : (ch + 1) * 256],
                            start=(ki == 0), stop=(ki == 2 * nch - 1))
                        ki += 1
                nc.scalar.activation(
                    outf[:, co * 256 : (co + 1) * 256],
                    ps[:], AF.Relu if relu else AF.Identity,
                    bias=bcol[:, bccol + co : bccol + co + 1])
            if outn is not None:
                for h in range(2):
                    ps = ps_med(128, 256)
                    ki = 0
                    for src in (xf, hf):
                        for ch in range(nch):
                            nc.tensor.matmul(
                                ps[:],
                                lhsT=src[:, ch * 256 + h * 128 : ch * 256 + (h + 1) * 128],
                                rhs=Wsb[:, ki * 256 : (ki + 1) * 256],
                                start=(ki == 0), stop=False)
                            ki += 1
                    nc.tensor.matmul(ps[:], lhsT=ones_at(rbias[0], 128).bitcast(F32),
                                     rhs=rrow(rbias, 256).bitcast(F32),
                                     start=False, stop=True)
                    nc.vector.tensor_scalar(outn[:, h * 256 : (h + 1) * 256], ps[:],
                                            0.0, None, op0=ALU.max)

        pool_hn(hp_nm, 768, hn1_fm)
        pool_lin(hp_fm, hn1_fm, 1536, qW1, BC_QB1, R_QB1, True, p1_fm, p1_nm)
        pool_hn(p1_nm, 256, hn2_fm)
        pool_lin(p1_fm, hn2_fm, 512, qW2, BC_QB2, R_QB2, True, p2_fm, p2_nm)
        pool_hn(p2_nm, 256, hn3_fm)
        pool_lin(p2_fm, hn3_fm, 512, qW3, BC_QB3, R_QB3, False, p3_fm, None)
        for L, pf in enumerate((p1_fm, p2_fm, p3_fm)):
            for co in range(2):
                for g in range(G):
                    nc.vector.tensor_reduce(
                        out_fm[:, (6 + L * 2 + co) * G + g : (6 + L * 2 + co) * G + g + 1],
                        pf[:, co * 256 + g * K : co * 256 + (g + 1) * K],
                        axis=AX.X, op=ALU.max)

        # ---------- final MLP ----------
        for co in range(2):
            ps = ps_sml(128, G)
            for k in range(12):
                nc.tensor.matmul(
                    ps[:], lhsT=mW1[:, k * 256 + co * 128 : k * 256 + co * 128 + 128],
                    rhs=out_fm[:, k * G : (k + 1) * G],
                    start=(k == 0), stop=(k == 11))
            nc.scalar.activation(y_sb[:, co * G : (co + 1) * G], ps[:], AF.Identity,
                                 bias=bcol[:, BC_MB1 + co : BC_MB1 + co + 1])
        zps = ps_sml(10, G)
        for ci in range(2):
            nc.tensor.matmul(zps[:], lhsT=mW2[:, ci * 10 : (ci + 1) * 10],
                             rhs=y_sb[:, ci * G : (ci + 1) * G],
                             start=(ci == 0), stop=(ci == 1))
        nc.scalar.activation(z_sb[:], zps[:], AF.Identity,
                             bias=bcol[0:10, BC_MB2 : BC_MB2 + 1])
        nc.sync.dma_start(yp_d[:], z_sb[:])

    nc.compile()
    return nc


# ---------------------------------------------------------------------------
# host side
# ---------------------------------------------------------------------------

def _pack_bcol(b):
    bc = np.zeros((128, BC_N), np.float32)
    for off, k in ((BC_B1, "b1"), (BC_B2, "b2"), (BC_B3, "b3"), (BC_AB1, "ab1"),
                   (BC_AB2, "ab2"), (BC_AB3, "ab3"), (BC_QB1, "qb1"),
                   (BC_QB2, "qb2"), (BC_QB3, "qb3"), (BC_MB1, "mb1")):
        v = np.asarray(b[k], np.float32)
        bc[:, off : off + v.size // 128] = v.reshape(-1, 128).T
    mb2 = np.asarray(b["mb2"], np.float32)
    bc[: mb2.size, BC_MB2] = mb2
    return bc


def tf32_round(v):
    u = np.ascontiguousarray(np.asarray(v, np.float32)).view(np.uint32).copy()
    u &= np.uint32(0xFFFFE000)
    return u.view(np.float32)


def _pack_rows(b, pb_lc):
    r = np.zeros((65, ROWS_W), np.float32)
    for p in (0, 32, 64):
        r[p, 0:512] = 1.0
    for (p, off), k in ((R_QB1, "qb1"), (R_QB2, "qb2"), (R_QB3, "qb3")):
        r[p, off : off + 256] = b[k]
    p, off = R_PB
    r[p, off : off + 256] = pb_lc
    return tf32_round(r)


def _edge_tables(edge_src, edge_dst, core):
    """Dedup'd (dst, count) tables per (src-tile, partition) for one core."""
    lo, hi = core * NLOC, (core + 1) * NLOC
    m = (edge_dst >= lo) & (edge_dst < hi)
    src = edge_src[m].astype(np.int64)
    dst = edge_dst[m].astype(np.int64)
    gg = dst // NPG
    if not np.array_equal(src // NPG, gg):
        raise ValueError("cross-graph edges break graph-parallel sharding")
    gl = gg - core * G
    sl = src - gg * NPG
    dl = dst - gg * NPG
    t = gl * 4 + sl // 128
    p = sl % 128
    key = (t * 128 + p) * NPG + dl
    uk, cnt = np.unique(key, return_counts=True)
    rows = uk // NPG
    cols = uk % NPG
    nrow = np.bincount(rows, minlength=T * 128)
    if nrow.max() > NI:
        raise ValueError(f"out-degree {nrow.max()} exceeds NI={NI}")
    starts = np.zeros(T * 128, np.int64)
    np.cumsum(nrow[:-1], out=starts[1:])
    pos = np.arange(uk.size) - starts[rows]
    at_idx = np.full((128, T * NI), -1, np.int16)
    at_val = np.zeros((128, T * NI), np.float32)
    pr = (rows % 128).astype(np.int64)
    tr = (rows // 128).astype(np.int64)
    at_idx[pr, tr * NI + pos] = cols.astype(np.int16)
    at_val[pr, tr * NI + pos] = cnt.astype(np.float32)
    return at_idx, at_val.astype(ml_dtypes.bfloat16)


_CACHE = {}
TRACE = False


def prepare_in_maps(inputs):
    f32 = lambda x: np.ascontiguousarray(np.asarray(x, np.float32))
    feat = f32(inputs["feat"])
    edge_src = np.asarray(inputs["edge_src"])
    edge_dst = np.asarray(inputs["edge_dst"])
    W = {k: tf32_round(inputs[k]) for k in
         ("W1", "W2", "W3", "aW1", "aW2", "aW3", "pW", "qW1", "qW2", "qW3",
          "mW1", "mW2")}
    b = {k: f32(inputs[k]) for k in
         ("b1", "b2", "b3", "ab1", "ab2", "ab3", "pb", "qb1", "qb2", "qb3",
          "mb1", "mb2")}
    ident = np.eye(128, dtype=np.float32)
    bcol = _pack_bcol(b)

    in_maps = []
    for c in range(NCORES):
        fs = feat[c * NLOC : (c + 1) * NLOC]
        feat_nm = np.ascontiguousarray(
            fs.reshape(T, 128, IN).transpose(1, 0, 2).reshape(128, T * IN))
        featT = np.ascontiguousarray(fs.T)
        at_idx, at_val = _edge_tables(edge_src, edge_dst, c)
        pW_lc = np.ascontiguousarray(W["pW"][:, c * G * K : (c + 1) * G * K])
        pb_lc = np.ascontiguousarray(b["pb"][c * G * K : (c + 1) * G * K])
        in_maps.append({
            "featT": tf32_round(featT), "feat_nm": tf32_round(feat_nm),
            "at_idx": at_idx, "at_val": at_val,
            "bcol": bcol, "rows2": _pack_rows(b, pb_lc),
            "ident": ident, "identr": ident,
            "W1": W["W1"], "W2": W["W2"], "W3": W["W3"],
            "aW1": W["aW1"], "aW2": W["aW2"], "aW3": W["aW3"],
            "pWl": pW_lc, "qW1": W["qW1"], "qW2": W["qW2"], "qW3": W["qW3"],
            "mW1": W["mW1"], "mW2": W["mW2"],
        })
    return in_maps


def kernel(**inputs):
    if "nc" not in _CACHE:
        _CACHE["nc"] = build_module()
    nc = _CACHE["nc"]
    in_maps = prepare_in_maps(inputs)
    res = run_bass_kernel_spmd(nc, in_maps, core_ids=list(range(NCORES)),
                               trace=TRACE)
    _CACHE["last_res"] = res
    out = np.zeros((B, 10), np.float32)
    for c in range(NCORES):
        out[c * G : (c + 1) * G, :] = np.asarray(res.results[c]["yp"]).T
    return out



# revision 52
# speedup vs baseline: 1.3054x; 1.3054x over previous
"""DiffPoolEncoder Trainium2 kernel.

Sharding: data parallel by graph. 8 cores x 4 graphs (512 nodes each).
Per core the whole network runs on-device; GraphSage aggregation uses dense
per-graph A^T tiles built on-device via gpsimd local_scatter from
host-prepared (dst, count) index tables (index-only preprocessing of the
edge list; clamped in-degrees are integer edge counts and ship as exact
f32). The h-branch (readout) runs in float32r; the assignment branch,
pooled stage and all spill-free node-major copies run in bfloat16 --
matmul row rate is identical, but bf16 halves SBUF/DMA and avoids the
f32r 4-cycles-per-row penalty on narrow outputs. Softmax, A@S, S^T X and
S^T A S are fused per-graph into the a3/logits loop so the pooled tail
overlaps the big assignment matmuls. SBUF pools are strictly LIFO-nested
by lifetime epoch.
"""

import sys

for _p in ("/opt/trn_rl_repo",):
    if _p not in sys.path:
        sys.path.append(_p)

import numpy as np
import ml_dtypes
from contextlib import ExitStack

import concourse.bass as bass
import concourse.mybir as mybir
import concourse.tile as tile
from concourse import bacc
from concourse.bass_utils import run_bass_kernel_spmd

F32 = mybir.dt.float32
F32R = mybir.dt.float32r
BF16 = mybir.dt.bfloat16
I16 = mybir.dt.int16
AF = mybir.ActivationFunctionType
ALU = mybir.AluOpType
AX = mybir.AxisListType

NCORES = 8
B = 32
NPG = 512
G = 4            # graphs per core
T = 16           # node tiles per core (4 per graph)
NLOC = 2048      # nodes per core
K = 64           # clusters per graph
IN = 128
HID = 256
NI = 48          # padded (dst,count) entries per (src-tile, partition)

# bcol column layout (each 128-chunk of a bias vector is one column)
BC_B1, BC_B2, BC_B3 = 0, 2, 4
BC_AB1, BC_AB2 = 6, 8
BC_AB3 = 10          # 16 cols
BC_QB1, BC_QB2, BC_QB3 = 26, 28, 30
BC_MB1, BC_MB2 = 32, 34
BC_PB = 35           # 4 cols: per-graph local pb, partitions 0:64
BC_N = 39

# rows2 [65, 1536] f32r: rows at matmul base partitions {0, 32, 64};
# ones[0:512] replicated at each used partition (matmul needs equal bases).
R_QB1 = (0, 512)
R_QB2, R_QB3 = (64, 512), (64, 768)
ROWS_W = 1024


def build_module():
    nc = bacc.Bacc("TRN2", target_bir_lowering=False)

    # ---------------- DRAM I/O ----------------
    featT_d = nc.dram_tensor("featT", [128, NLOC], F32R, kind="ExternalInput")
    featnm_d = nc.dram_tensor("feat_nm", [128, T * IN], F32R, kind="ExternalInput")
    atidx_d = nc.dram_tensor("at_idx", [128, T * NI], I16, kind="ExternalInput")
    atval_d = nc.dram_tensor("at_val", [128, T * NI], BF16, kind="ExternalInput")
    bcol_d = nc.dram_tensor("bcol", [128, BC_N], F32, kind="ExternalInput")
    rows_d = nc.dram_tensor("rows2", [65, ROWS_W], F32R, kind="ExternalInput")
    ident_d = nc.dram_tensor("ident", [128, 128], F32, kind="ExternalInput")
    identr_d = nc.dram_tensor("identr", [128, 128], F32R, kind="ExternalInput")
    # clamped in-degree (exact integer counts), per-graph gslot rows
    degr_d = nc.dram_tensor("degr", [65, 1024], F32R, kind="ExternalInput")
    w_d = {}
    for name, fi, fo, dt in [
        ("W1", 256, 256, F32R), ("W2", 512, 256, F32R), ("W3", 512, 256, F32R),
        ("aW1", 256, 256, F32R), ("aW2", 512, 256, BF16),
        ("aW3", 512, 2048, BF16), ("pWl", 2560, 256, BF16),
        ("qW1", 1536, 256, BF16), ("qW2", 512, 256, BF16),
        ("qW3", 512, 256, BF16), ("mW1", 1536, 256, F32R),
        ("mW2", 256, 10, F32R),
    ]:
        w_d[name] = nc.dram_tensor(name, [fi, fo], dt, kind="ExternalInput")
    yp_d = nc.dram_tensor("yp", [10, G], F32, kind="ExternalOutput")

    with tile.TileContext(nc) as tc, ExitStack() as ex, \
            nc.allow_low_precision(reason="f32r/bf16 matmuls; accumulation stays fp32 in PSUM"):
        persist = ex.enter_context(tc.tile_pool(name="persist", bufs=1))
        # PSUM: 8 banks. One tag per pool so slot count == bank count.
        ps_p = ex.enter_context(tc.tile_pool(name="psP", bufs=3, space="PSUM"))
        lg_p = ex.enter_context(tc.tile_pool(name="psL", bufs=1, space="PSUM"))
        pm_p = ex.enter_context(tc.tile_pool(name="psM", bufs=2, space="PSUM"))
        pl_p = ex.enter_context(tc.tile_pool(name="psS", bufs=2, space="PSUM"))

        uid = [0]

        def _nm(pfx):
            uid[0] += 1
            return f"{pfx}{uid[0]}"

        def ps_big(dt=F32):
            return ps_p.tile([128, 512], dt, tag="ps", name=_nm("ps"))

        def ps_med(p, f, dt=F32):
            return pm_p.tile([p, f], dt, tag="pm", name=_nm("pm"))

        def ps_sml(p, f, dt=F32):
            return pl_p.tile([p, f], dt, tag="pl", name=_nm("pl"))

        def wload(pool, name, fi, fo, dt=F32R):
            kk = fi // 128
            sb = pool.tile([128, kk * fo], dt, tag=name, name=name)
            nc.sync.dma_start(
                sb[:].rearrange("p (k f) -> p k f", k=kk, f=fo),
                w_d[name][:, :].rearrange("(k p) f -> p k f", p=128),
            )
            return sb

        # ---------- persistent small tensors (epoch E6) ----------
        ident = persist.tile([128, 128], F32)
        identr = persist.tile([128, 128], F32R)
        identb = persist.tile([128, 128], BF16)
        bcol = persist.tile([128, BC_N], F32)
        ones_pr = persist.tile([65, 128], F32R)  # ones rows at base partitions
        nmax = persist.tile([128, 2], F32)
        sumx = persist.tile([128, 2], F32)
        nc.vector.memset(ones_pr[:].bitcast(F32), 1.0)

        def ones_at(p, n):
            return ones_pr[p : p + 1, 0:n]

        # right-side stack: tensors produced mid-stream and consumed by the
        # pooled tail; outlives the left-side phase pools.
        keep = ex.enter_context(tc.tile_pool(name="keep", bufs=1, side="right"))
        out_fm = keep.tile([128, 12 * G], F32R)  # readout maxes, col=ch*G+g
        y_sb = keep.tile([128, 2 * G], F32R)
        z_sb = keep.tile([10, G], F32)
        # node-major g_emb layers, bf16, for the pooled stage (no DRAM spill)
        h1nb = keep.tile([128, T * HID], BF16)
        h2nb = keep.tile([128, T * HID], BF16)

        # AT (scaled A^T tiles, f32r) lives until AS (epoch E6)
        at_p = ex.enter_context(tc.tile_pool(name="atp", bufs=1))
        AT = at_p.tile([128, T * NPG], F32R)

        # ---------- LIFO phase pools ----------
        ex5 = ExitStack()   # close after logits/softmax
        agg_p = ex5.enter_context(tc.tile_pool(name="aggfm", bufs=1))
        afm1_p = ex5.enter_context(tc.tile_pool(name="afm1", bufs=1))
        afm2_p = ex5.enter_context(tc.tile_pool(name="afm2", bufs=1))
        ex3 = ExitStack()   # close after h3 (h3 runs after xnm closes)
        xfm_p = ex3.enter_context(tc.tile_pool(name="xfm", bufs=2))
        w3_p = ex3.enter_context(tc.tile_pool(name="w3p", bufs=1))
        ex4 = ExitStack()   # close after agg_a2
        xnm_p = ex4.enter_context(tc.tile_pool(name="xnm", bufs=2))
        ex2c = ExitStack()  # close after a2
        a2w_p = ex2c.enter_context(tc.tile_pool(name="a2wp", bufs=1))
        ex2b = ExitStack()  # close after h2
        w2_p = ex2b.enter_context(tc.tile_pool(name="w2p", bufs=1))
        ex2 = ExitStack()   # close after h1/a1
        w1_p = ex2.enter_context(tc.tile_pool(name="w1p", bufs=1))
        agf_p = ex2.enter_context(tc.tile_pool(name="agfp", bufs=1))
        ex1 = ExitStack()   # close after agg_feat
        fnm_p = ex1.enter_context(tc.tile_pool(name="fnmp", bufs=1))
        ex0 = ExitStack()   # close after A^T built+scaled
        ate_p = ex0.enter_context(tc.tile_pool(name="atep", bufs=1))

        # ---------- input DMAs ----------
        atbf_idx = ate_p.tile([128, T * NI], I16, tag="atidx")
        atbf_val = ate_p.tile([128, T * NI], BF16, tag="atval")
        # DMA priority order: the scatter/scale chain first, then layer-1
        # activations, then constants and weights.
        dinvr = ate_p.tile([65, 1024], F32R, tag="dinvr")  # deg -> 1/deg rows
        nc.sync.dma_start(atbf_idx[:], atidx_d[:])
        nc.sync.dma_start(atbf_val[:], atval_d[:])
        nc.sync.dma_start(dinvr[:], degr_d[:])
        featnm = fnm_p.tile([128, T * IN], F32R, tag="featnm")
        nc.sync.dma_start(featnm[:], featnm_d[:])
        ftg = []
        for g in range(G):
            ft = fnm_p.tile([128, NPG], F32R, tag="ftc", name=_nm("ftc"),
                            bufs=2)
            nc.sync.dma_start(ft[:], featT_d[:, g * NPG : (g + 1) * NPG])
            ftg.append(ft)
        nc.sync.dma_start(ident[:], ident_d[:])
        nc.sync.dma_start(identr[:], identr_d[:])
        nc.sync.dma_start(bcol[:], bcol_d[:])
        W1 = wload(w1_p, "W1", 256, 256)
        aW1 = wload(w1_p, "aW1", 256, 256)
        W2 = wload(w2_p, "W2", 512, 256)
        W3 = wload(w3_p, "W3", 512, 256)
        aW2 = wload(a2w_p, "aW2", 512, 256, BF16)

        gslot = [(0, 0), (32, 0), (64, 0), (32, 512)]
        for g in range(G):
            p, off = gslot[g]
            nc.vector.reciprocal(dinvr[p : p + 1, off : off + NPG],
                                 dinvr[p : p + 1, off : off + NPG])

        # ---------- emit helpers ----------
        def emit_agg(x_nm, D, out_t, glist=range(G)):
            """out_t[d, n] (feature-major) = sum_s x_nm[s, d] * AT[s, n]."""
            for g in glist:
                for ch in range(D // 128):
                    ps = ps_big()
                    for st in range(4):
                        t = g * 4 + st
                        nc.tensor.matmul(
                            ps[:],
                            lhsT=x_nm[:, t * D + ch * 128 : t * D + ch * 128 + 128],
                            rhs=AT[:, t * NPG : (t + 1) * NPG],
                            start=(st == 0), stop=(st == 3))
                    # AT holds raw edge counts; the mean-normalization by
                    # 1/deg[dst] rides along with the DVE evac for free
                    nc.vector.tensor_tensor(
                        out=out_t[:, ch * NLOC + g * NPG : ch * NLOC + (g + 1) * NPG],
                        in0=ps[:], in1=dbc[:, g * NPG : (g + 1) * NPG],
                        op=ALU.mult)

        def emit_lin_fm(x_fm, a_fm, Din, Dout, Wsb, bccol, relu, out_t,
                        nblist=range(4)):
            nk = Din // 128
            for co in range(Dout // 128):
                for nb in nblist:
                    ps = ps_big()
                    ki = 0
                    for src in (x_fm, a_fm):
                        for ci in range(nk):
                            nc.tensor.matmul(
                                ps[:],
                                lhsT=Wsb[:, ki * Dout + co * 128 : ki * Dout + co * 128 + 128],
                                rhs=src[:, ci * NLOC + nb * 512 : ci * NLOC + (nb + 1) * 512],
                                start=(ki == 0), stop=(ki == 2 * nk - 1))
                            ki += 1
                    nc.scalar.activation(
                        out_t[:, co * NLOC + nb * 512 : co * NLOC + (nb + 1) * 512],
                        ps[:], AF.Relu if relu else AF.Identity,
                        bias=bcol[:, bccol + co : bccol + co + 1])

        def emit_nm_T(x_fm, out_nm, dt=F32R, bcopy=None, t2list=None):
            # node-major via PE transposes of the (already relu'd) fm tensor;
            # 4 transposed blocks share one PSUM bank -> single 512-wide evac.
            # bcopy: persistent bf16 copy evacuated on the idle Pool engine.
            idm = identb if dt == BF16 else identr
            for t2 in (range(0, T, 2) if t2list is None else t2list):
                tp = ps_big(dt)
                for i, (t, ch) in enumerate(
                        ((t2, 0), (t2, 1), (t2 + 1, 0), (t2 + 1, 1))):
                    nc.tensor.matmul(
                        tp[:, i * 128 : (i + 1) * 128],
                        lhsT=x_fm[:, ch * NLOC + t * 128 : (ch * NLOC + t * 128) + 128],
                        rhs=idm[:], is_transpose=True,
                        start=True, stop=True, skip_group_check=True)
                sl = slice(t2 * HID, (t2 + 2) * HID)
                if out_nm is not None:
                    if dt == BF16:
                        nc.scalar.copy(out_nm[:, sl], tp[:])
                    else:
                        nc.vector.tensor_copy(out_nm[:, sl], tp[:])
                    if bcopy is not None:
                        # bf16 shadow for the pooled tail; SBUF->SBUF on the
                        # idle Pool engine (GPSIMD cannot read PSUM)
                        nc.gpsimd.tensor_copy(bcopy[:, sl], out_nm[:, sl])
                elif bcopy is not None:
                    nc.scalar.copy(bcopy[:, sl], tp[:])

        def emit_out1(x_fm, ch0, glist=range(G)):
            for ci in range(2):
                for g in glist:
                    nc.vector.tensor_reduce(
                        out_fm[:, (ch0 + ci) * G + g : (ch0 + ci) * G + g + 1],
                        x_fm[:, ci * NLOC + g * NPG : ci * NLOC + (g + 1) * NPG],
                        axis=AX.X, op=ALU.max)

        # ---------- phase 0 fused with h1/a1, pipelined per graph ----------
        aggfeat = agf_p.tile([128, NLOC], F32R, tag="aggf", name="aggfeat")
        h1f = xfm_p.tile([128, 2 * NLOC], F32R, tag="xfm", name="h1f")
        h1n = xnm_p.tile([128, T * HID], F32R, tag="xnm", name="h1n")
        a1f = afm1_p.tile([128, 2 * NLOC], BF16, tag="a1f", name="a1f")
        a1n = xnm_p.tile([128, T * HID], F32R, tag="xnm", name="a1n")

        def emit_lin1(ftc, g, Wsb, bccol, out_t):
            # layer-1 linear for one graph: cat(feat, agg_feat) @ W
            for co in range(2):
                ps = ps_big()
                nc.tensor.matmul(
                    ps[:], lhsT=Wsb[:, co * 128 : co * 128 + 128],
                    rhs=ftc[:], start=True, stop=False)
                nc.tensor.matmul(
                    ps[:], lhsT=Wsb[:, 256 + co * 128 : 256 + co * 128 + 128],
                    rhs=aggfeat[:, g * NPG : (g + 1) * NPG],
                    start=False, stop=True)
                nc.scalar.activation(
                    out_t[:, co * NLOC + g * NPG : co * NLOC + (g + 1) * NPG],
                    ps[:], AF.Relu,
                    bias=bcol[:, bccol + co : bccol + co + 1])

        # A^T build first for ALL graphs: keeps the in-order Pool queue
        # (scatters) free of later-phase ops, so graph g+1's scatter never
        # waits behind graph g's h1 consumers. AT keeps RAW counts; the
        # 1/deg broadcast tiles (dbc) are applied at every agg evac instead.
        dbc = agg_p.tile([128, G * NPG], F32R, tag="dbc", name="dbc")
        for g in range(G):
            p, off = gslot[g]
            bps = ps_big()
            nc.tensor.matmul(bps[:], lhsT=ones_at(p, 128),
                             rhs=dinvr[p : p + 1, off : off + NPG],
                             start=True, stop=True)
            nc.scalar.copy(dbc[:, g * NPG : (g + 1) * NPG], bps[:])
        for t in range(T):
            scr = ate_p.tile([128, NPG], BF16, tag="scscr", name=_nm("sc"),
                             bufs=2)
            nc.gpsimd.local_scatter(
                out_ap=scr[:],
                data_ap=atbf_val[:, t * NI : (t + 1) * NI],
                idxs_ap=atbf_idx[:, t * NI : (t + 1) * NI],
                channels=128, num_elems=NPG, num_idxs=NI,
            )
            # raw A^T tile; alternate evac engine to balance Act/DVE
            if t % 2 == 0:
                nc.scalar.copy(AT[:, t * NPG : (t + 1) * NPG], scr[:])
            else:
                nc.vector.tensor_copy(AT[:, t * NPG : (t + 1) * NPG], scr[:])
        nc.vector.tensor_copy(identb[:], ident[:])
        for g in range(G):
            emit_agg(featnm, IN, aggfeat, glist=[g])
            emit_lin1(ftg[g], g, W1, BC_B1, h1f)
            emit_lin1(ftg[g], g, aW1, BC_AB1, a1f)
            emit_nm_T(h1f, h1n, bcopy=h1nb, t2list=(4 * g, 4 * g + 2))
            emit_nm_T(a1f, a1n, dt=BF16, t2list=(4 * g, 4 * g + 2))
        emit_out1(h1f, 0)
        ex0.close()
        ex1.close()
        ex2.close()

        aggh1 = agg_p.tile([128, 2 * NLOC], F32R, tag="aggh", name="aggh1")
        emit_agg(h1n, HID, aggh1)

        agga1 = agg_p.tile([128, 2 * NLOC], BF16, tag="agga", name="agga1")
        emit_agg(a1n, HID, agga1)

        h2f = xfm_p.tile([128, 2 * NLOC], F32R, tag="xfm", name="h2f")
        h2n = xnm_p.tile([128, T * HID], F32R, tag="xnm", name="h2n")
        emit_lin_fm(h1f, aggh1, 256, 256, W2, BC_B2, True, h2f)
        emit_nm_T(h2f, h2n, bcopy=h2nb)
        emit_out1(h2f, 2)
        ex2b.close()

        a2f = afm2_p.tile([128, 2 * NLOC], BF16, tag="a2f", name="a2f")
        a2n = xnm_p.tile([128, T * HID], F32R, tag="xnm", name="a2n")
        emit_lin_fm(a1f, agga1, 256, 256, aW2, BC_AB2, True, a2f)
        emit_nm_T(a2f, a2n, dt=BF16)
        ex2c.close()

        aggh2 = agg_p.tile([128, 2 * NLOC], F32R, tag="aggh", name="aggh2")
        emit_agg(h2n, HID, aggh2)

        agga2 = agg_p.tile([128, 2 * NLOC], BF16, tag="agga", name="agga2")
        emit_agg(a2n, HID, agga2)
        ex4.close()

        # pooled-stage tiles + weights: prefetch during the a3/logits phase.
        # Right-side SBUF stack: outlives the left-side phase pools without
        # violating their LIFO discipline.
        late = ex.enter_context(tc.tile_pool(name="late", bufs=1, side="right"))
        rows2 = late.tile([65, ROWS_W], F32R, tag="rows2", name="rows2")
        nc.sync.dma_start(rows2[:], rows_d[:])

        def rrow(ro, n):
            p, off = ro
            return rows2[p : p + 1, off : off + n]

        qW1 = wload(late, "qW1", 1536, 256, BF16)
        qW2 = wload(late, "qW2", 512, 256, BF16)
        qW3 = wload(late, "qW3", 512, 256, BF16)
        S_b = late.tile([128, T * K], BF16, tag="S_b", name="S_b")
        h3nb = late.tile([128, T * HID], BF16, tag="h3nb", name="h3nb")
        AS_nm = late.tile([128, T * K], BF16, tag="AS", name="AS_nm")
        hp_nm = late.tile([128, 2 * 768], BF16, tag="hpn", name="hp_nm")
        hp_fm = late.tile([128, 6 * 256], BF16, tag="hpf", name="hp_fm")
        adjg = late.tile([128, 2 * K], F32, tag="adjg", name="adjg")
        rsum = late.tile([128, 2], F32, tag="rsum", name="rsum")
        adjT = late.tile([128, 2 * 128], BF16, tag="adjT", name="adjT")
        hn1_fm = late.tile([128, 6 * 256], BF16, tag="hn1", name="hn1_fm")
        p1_nm = late.tile([128, 2 * 256], BF16, tag="p1n", name="p1_nm")
        p1_fm = late.tile([128, 2 * 256], BF16, tag="p1f", name="p1_fm")
        hn2_fm = late.tile([128, 2 * 256], BF16, tag="hn2", name="hn2_fm")
        p2_nm = late.tile([128, 2 * 256], BF16, tag="p2n", name="p2_nm")
        p2_fm = late.tile([128, 2 * 256], BF16, tag="p2f", name="p2_fm")
        hn3_fm = late.tile([128, 2 * 256], BF16, tag="hn3", name="hn3_fm")
        p3_fm = late.tile([128, 2 * 256], BF16, tag="p3f", name="p3_fm")
        nc.vector.memset(adjT[:].bitcast(F32), 0.0)
        Xr = [h1nb, h2nb, h3nb]

        # h3: fm + readout; node-major -> persistent bf16 only
        h3f = xfm_p.tile([128, 2 * NLOC], F32R, tag="xfm", name="h3f")
        emit_lin_fm(h2f, aggh2, 256, 256, W3, BC_B3, False, h3f)
        emit_nm_T(h3f, None, bcopy=h3nb)
        emit_out1(h3f, 4)
        ex3.close()

        # a3/logits weights on the right stack, opened once xfm/w3 are gone
        ex5b = ExitStack()
        wl_p = ex5b.enter_context(tc.tile_pool(name="wlate", bufs=1,
                                               side="right"))
        aW3 = wl_p.tile([128, 4 * 2048], BF16, tag="aW3", name="aW3")
        aW3v = aW3[:].rearrange("p (k f) -> p k f", k=4, f=2048)
        for q in range(4):
            nc.sync.dma_start(
                aW3v[:, :, q * 512 : (q + 1) * 512],
                w_d["aW3"][:, q * 512 : (q + 1) * 512].rearrange(
                    "(k p) f -> p k f", p=128))
        pWl = wload(wl_p, "pWl", 2560, 256, BF16)

        # ---------- a3 + logits + softmax + AS + pooled prep, per graph ----
        def emit_hpool(h):
            for L in range(3):
                for gs in range(2):
                    g = h * 2 + gs
                    ps = ps_med(64, 256)
                    for j in range(4):
                        t = g * 4 + j
                        nc.tensor.matmul(
                            ps[:],
                            lhsT=S_b[:, t * K : (t + 1) * K],
                            rhs=Xr[L][:, t * HID : (t + 1) * HID],
                            start=(j == 0), stop=(j == 3))
                    dst = hp_nm[gs * 64 : gs * 64 + 64,
                                h * 768 + L * 256 : h * 768 + (L + 1) * 256]
                    if gs == 0:
                        nc.vector.tensor_copy(dst, ps[:])
                    else:
                        sh = late.tile([64, 256], BF16, tag="hpsh",
                                       name=_nm("hpsh"), bufs=2)
                        nc.vector.tensor_copy(sh[:], ps[:])
                        nc.sync.dma_start(dst, sh[:])
            for ch in range(6):  # hp_fm via transposes of the pair tile
                tp = ps_med(128, 128, BF16)
                nc.tensor.matmul(
                    tp[:], lhsT=hp_nm[:, h * 768 + ch * 128 : h * 768 + (ch + 1) * 128],
                    rhs=identb[:], is_transpose=True,
                    start=True, stop=True, skip_group_check=True)
                nc.vector.tensor_copy(
                    hp_fm[:, ch * 256 + h * 128 : ch * 256 + (h + 1) * 128], tp[:])

        def emit_adj(h):
            for gs in range(2):
                g = h * 2 + gs
                ps = ps_sml(64, K)
                for j in range(4):
                    t = g * 4 + j
                    nc.tensor.matmul(ps[:],
                                     lhsT=S_b[:, t * K : (t + 1) * K],
                                     rhs=AS_nm[:, t * K : (t + 1) * K],
                                     start=(j == 0), stop=(j == 3))
                dst = adjg[gs * 64 : gs * 64 + 64, h * K : (h + 1) * K]
                if gs == 0:
                    nc.vector.tensor_copy(dst, ps[:])
                else:
                    sh = late.tile([64, K], F32, tag="adsh",
                                   name=_nm("adsh"), bufs=2)
                    nc.vector.tensor_copy(sh[:], ps[:])
                    nc.sync.dma_start(dst, sh[:])
            nc.vector.tensor_reduce(rsum[:, h : h + 1], adjg[:, h * K : (h + 1) * K],
                                    axis=AX.X, op=ALU.add)
            nc.vector.tensor_scalar(rsum[:, h : h + 1], rsum[:, h : h + 1],
                                    1e-9, None, op0=ALU.add)
            nc.vector.reciprocal(rsum[:, h : h + 1], rsum[:, h : h + 1])
            nc.vector.tensor_scalar(adjg[:, h * K : (h + 1) * K],
                                    adjg[:, h * K : (h + 1) * K],
                                    rsum[:, h : h + 1], None, op0=ALU.mult)
            # transpose each graph's [64,64] block onto the block diagonal
            # (transpose outputs must land at PSUM partition 0; odd block is
            # partition-shifted into place with a small SBUF->SBUF DMA)
            for gs in range(2):
                tp = ps_sml(128, K)
                nc.tensor.transpose(
                    tp[0:64, :],
                    adjg[gs * 64 : gs * 64 + 64, h * K : (h + 1) * K],
                    ident[gs * 64 : gs * 64 + 64, gs * 64 : gs * 64 + 64]
                    if gs else ident[0:64, 0:64])
                if gs == 0:
                    nc.vector.tensor_copy(adjT[0:64, h * 128 : h * 128 + 64],
                                          tp[0:64, :])
                else:
                    sb = late.tile([64, K], BF16, tag="adjsh", name=_nm("adjsh"),
                                   bufs=2)
                    nc.vector.tensor_copy(sb[:], tp[0:64, :])
                    nc.sync.dma_start(
                        adjT[64:128, h * 128 + 64 : h * 128 + 128], sb[:])

        def pool_hn_h(x_nm, xw, out_t, h):
            # out[d, u] = sum_v x_nm[v, d] * adjT_bd[v, u], one graph pair
            for ch in range(xw // 128):
                tp = ps_sml(128, 128)
                nc.tensor.matmul(
                    tp[:],
                    lhsT=x_nm[:, h * xw + ch * 128 : h * xw + (ch + 1) * 128],
                    rhs=adjT[:, h * 128 : (h + 1) * 128],
                    start=True, stop=True)
                nc.vector.tensor_copy(
                    out_t[:, ch * 256 + h * 128 : ch * 256 + (h + 1) * 128],
                    tp[:])

        def pool_lin_h(xf, hf, Din, Wsb, bccol, rbias, relu, ch0, outf, outn,
                       h):
            nch = Din // 256
            for co in range(2):
                ps = ps_med(128, 128)
                ki = 0
                for src in (xf, hf):
                    for ch in range(nch):
                        nc.tensor.matmul(
                            ps[:],
                            lhsT=Wsb[:, ki * 256 + co * 128 : ki * 256 + co * 128 + 128],
                            rhs=src[:, ch * 256 + h * 128 : ch * 256 + (h + 1) * 128],
                            start=(ki == 0), stop=(ki == 2 * nch - 1))
                        ki += 1
                nc.scalar.activation(
                    outf[:, co * 256 + h * 128 : co * 256 + (h + 1) * 128],
                    ps[:], AF.Relu if relu else AF.Identity,
                    bias=bcol[:, bccol + co : bccol + co + 1])
                for gs in range(2):
                    g2 = 2 * h + gs
                    nc.vector.tensor_reduce(
                        out_fm[:, (ch0 + co) * G + g2 : (ch0 + co) * G + g2 + 1],
                        outf[:, co * 256 + h * 128 + gs * K : co * 256 + h * 128 + (gs + 1) * K],
                        axis=AX.X, op=ALU.max)
            if outn is not None:
                ps = ps_med(128, 256)
                ki = 0
                for src in (xf, hf):
                    for ch in range(nch):
                        nc.tensor.matmul(
                            ps[:],
                            lhsT=src[:, ch * 256 + h * 128 : ch * 256 + (h + 1) * 128],
                            rhs=Wsb[:, ki * 256 : (ki + 1) * 256],
                            start=(ki == 0), stop=False)
                        ki += 1
                nc.tensor.matmul(ps[:], lhsT=ones_at(rbias[0], 128).bitcast(F32),
                                 rhs=rrow(rbias, 256).bitcast(F32),
                                 start=False, stop=True)
                nc.vector.tensor_scalar(outn[:, h * 256 : (h + 1) * 256], ps[:],
                                        0.0, None, op0=ALU.max)

        for g in range(G):
            lps = lg_p.tile([64, 512], F32, tag="lg", name=_nm("lg"))
            for co in range(16):  # a3 = relu(cat(a2, agg_a2) @ aW3 + ab3)
                ps3 = ps_big()
                ki = 0
                for src in (a2f, agga2):
                    for ci in range(2):
                        nc.tensor.matmul(
                            ps3[:],
                            lhsT=aW3[:, ki * 2048 + co * 128 : ki * 2048 + co * 128 + 128],
                            rhs=src[:, ci * NLOC + g * NPG : ci * NLOC + (g + 1) * NPG],
                            start=(ki == 0), stop=(ki == 3))
                        ki += 1
                a3b = wl_p.tile([128, 512], BF16, tag="a3buf", name=_nm("a3b"), bufs=2)
                nc.scalar.activation(a3b[:], ps3[:], AF.Relu,
                                     bias=bcol[:, BC_AB3 + co : BC_AB3 + co + 1])
                nc.tensor.matmul(
                    lps[:], lhsT=pWl[:, (4 + co) * 256 + g * K : (4 + co) * 256 + g * K + K],
                    rhs=a3b[:], start=(co == 0), stop=False)
            for ci in range(2):  # a1 block of pW
                nc.tensor.matmul(
                    lps[:], lhsT=pWl[:, ci * 256 + g * K : ci * 256 + g * K + K],
                    rhs=a1f[:, ci * NLOC + g * NPG : ci * NLOC + (g + 1) * NPG],
                    start=False, stop=False)
            for ci in range(2):  # a2 block
                nc.tensor.matmul(
                    lps[:], lhsT=pWl[:, (2 + ci) * 256 + g * K : (2 + ci) * 256 + g * K + K],
                    rhs=a2f[:, ci * NLOC + g * NPG : ci * NLOC + (g + 1) * NPG],
                    start=False, stop=(ci == 1))
            lgf = wl_p.tile([64, 512], F32, tag="lgf", name=_nm("lgf"), bufs=1)
            nc.scalar.activation(lgf[:], lps[:], AF.Identity,
                                 bias=bcol[0:64, BC_PB + g : BC_PB + g + 1])
            lgt = wl_p.tile([128, 4 * K], F32, tag="lgs", name=_nm("lgs"),
                            bufs=2)
            for j in range(4):  # transpose to node-major
                tps = ps_med(128, 64)
                nc.tensor.transpose(tps[:], lgf[0:64, j * 128 : (j + 1) * 128],
                                    ident[0:64, 0:64])
                nc.vector.tensor_copy(lgt[:, j * K : (j + 1) * K], tps[:])

            # masked softmax == per-graph softmax over K columns. S kept in
            # two forms: transient f32r (for the f32r AS matmuls against AT)
            # and persistent bf16 (for the bf16 pooled stage) — the compiler
            # rejects 32-bit x 16-bit matmul operand mixes.
            S32 = wl_p.tile([128, 4 * K], F32R, tag="S32", name=_nm("S32"),
                            bufs=2)
            for j in range(4):
                t = g * 4 + j
                bb = t % 2
                nc.vector.tensor_reduce(nmax[:, bb : bb + 1],
                                        lgt[:, j * K : (j + 1) * K],
                                        axis=AX.X, op=ALU.max, negate=True)
                nc.scalar.activation(S32[:, j * K : (j + 1) * K],
                                     lgt[:, j * K : (j + 1) * K], AF.Exp,
                                     bias=nmax[:, bb : bb + 1],
                                     accum_out=sumx[:, bb : bb + 1])
                nc.vector.reciprocal(sumx[:, bb : bb + 1], sumx[:, bb : bb + 1])
                nc.vector.tensor_scalar(S32[:, j * K : (j + 1) * K],
                                        S32[:, j * K : (j + 1) * K],
                                        sumx[:, bb : bb + 1], None, op0=ALU.mult)
                nc.gpsimd.tensor_copy(S_b[:, t * K : (t + 1) * K],
                                      S32[:, j * K : (j + 1) * K])

            # AS = A @ S: scaled-AT product un-scaled by clamped deg (exact)
            for j in range(4):
                t = g * 4 + j
                ps = ps_sml(128, K)
                for st in range(4):
                    nc.tensor.matmul(
                        ps[:],
                        lhsT=AT[:, (g * 4 + st) * NPG + j * 128 : (g * 4 + st) * NPG + (j + 1) * 128],
                        rhs=S32[:, st * K : (st + 1) * K],
                        start=(st == 0), stop=(st == 3))
                nc.vector.tensor_copy(AS_nm[:, t * K : (t + 1) * K], ps[:])
            if g % 2 == 1:  # pooled stack for the completed pair
                h = g // 2
                emit_hpool(h)
                emit_adj(h)
                pool_hn_h(hp_nm, 768, hn1_fm, h)
                pool_lin_h(hp_fm, hn1_fm, 1536, qW1, BC_QB1, R_QB1, True, 6,
                           p1_fm, p1_nm, h)
                pool_hn_h(p1_nm, 256, hn2_fm, h)
                pool_lin_h(p1_fm, hn2_fm, 512, qW2, BC_QB2, R_QB2, True, 8,
                           p2_fm, p2_nm, h)
                pool_hn_h(p2_nm, 256, hn3_fm, h)
                pool_lin_h(p2_fm, hn3_fm, 512, qW3, BC_QB3, R_QB3, False, 10,
                           p3_fm, None, h)

        ex5b.close()
        ex5.close()

        late2 = ex.enter_context(tc.tile_pool(name="late2", bufs=1, side="right"))
        mW1 = wload(late2, "mW1", 1536, 256)
        mW2 = wload(late2, "mW2", 256, 10)

        # ---------- final MLP ----------
        for co in range(2):
            ps = ps_sml(128, G)
            for k in range(12):
                nc.tensor.matmul(
                    ps[:], lhsT=mW1[:, k * 256 + co * 128 : k * 256 + co * 128 + 128],
                    rhs=out_fm[:, k * G : (k + 1) * G],
                    start=(k == 0), stop=(k == 11))
            nc.scalar.activation(y_sb[:, co * G : (co + 1) * G], ps[:], AF.Identity,
                                 bias=bcol[:, BC_MB1 + co : BC_MB1 + co + 1])
        zps = ps_sml(10, G)
        for ci in range(2):
            nc.tensor.matmul(zps[:], lhsT=mW2[:, ci * 10 : (ci + 1) * 10],
                             rhs=y_sb[:, ci * G : (ci + 1) * G],
                             start=(ci == 0), stop=(ci == 1))
        nc.scalar.activation(z_sb[:], zps[:], AF.Identity,
                             bias=bcol[0:10, BC_MB2 : BC_MB2 + 1])
        nc.sync.dma_start(yp_d[:], z_sb[:])

    nc.compile()
    return nc


# ---------------------------------------------------------------------------
# host side
# ---------------------------------------------------------------------------

def _pack_bcol(b, pb_lc):
    bc = np.zeros((128, BC_N), np.float32)
    for off, k in ((BC_B1, "b1"), (BC_B2, "b2"), (BC_B3, "b3"), (BC_AB1, "ab1"),
                   (BC_AB2, "ab2"), (BC_AB3, "ab3"), (BC_QB1, "qb1"),
                   (BC_QB2, "qb2"), (BC_QB3, "qb3"), (BC_MB1, "mb1")):
        v = np.asarray(b[k], np.float32)
        bc[:, off : off + v.size // 128] = v.reshape(-1, 128).T
    mb2 = np.asarray(b["mb2"], np.float32)
    bc[: mb2.size, BC_MB2] = mb2
    bc[0:64, BC_PB : BC_PB + G] = pb_lc.reshape(G, K).T
    return bc


def tf32_round(v):
    u = np.ascontiguousarray(np.asarray(v, np.float32)).view(np.uint32).copy()
    u &= np.uint32(0xFFFFE000)
    return u.view(np.float32)


def _pack_rows(b):
    r = np.zeros((65, ROWS_W), np.float32)
    for p in (0, 32, 64):
        r[p, 0:512] = 1.0
    for (p, off), k in ((R_QB1, "qb1"), (R_QB2, "qb2"), (R_QB3, "qb3")):
        r[p, off : off + 256] = b[k]
    return tf32_round(r)


def _edge_tables(edge_src, edge_dst, core):
    """Dedup'd (dst, count) tables per (src-tile, partition) for one core."""
    lo, hi = core * NLOC, (core + 1) * NLOC
    m = (edge_dst >= lo) & (edge_dst < hi)
    src = edge_src[m].astype(np.int64)
    dst = edge_dst[m].astype(np.int64)
    gg = dst // NPG
    if not np.array_equal(src // NPG, gg):
        raise ValueError("cross-graph edges break graph-parallel sharding")
    gl = gg - core * G
    sl = src - gg * NPG
    dl = dst - gg * NPG
    t = gl * 4 + sl // 128
    p = sl % 128
    key = (t * 128 + p) * NPG + dl
    uk, cnt = np.unique(key, return_counts=True)
    rows = uk // NPG
    cols = uk % NPG
    nrow = np.bincount(rows, minlength=T * 128)
    if nrow.max() > NI:
        raise ValueError(f"out-degree {nrow.max()} exceeds NI={NI}")
    starts = np.zeros(T * 128, np.int64)
    np.cumsum(nrow[:-1], out=starts[1:])
    pos = np.arange(uk.size) - starts[rows]
    at_idx = np.full((128, T * NI), -1, np.int16)
    at_val = np.zeros((128, T * NI), np.float32)
    pr = (rows % 128).astype(np.int64)
    tr = (rows // 128).astype(np.int64)
    at_idx[pr, tr * NI + pos] = cols.astype(np.int16)
    at_val[pr, tr * NI + pos] = cnt.astype(np.float32)
    # clamped in-degree per local node (exact integer counts)
    degl = np.bincount(dst - lo, minlength=NLOC).astype(np.float32)
    degl = np.maximum(degl, 1.0)
    return at_idx, at_val.astype(ml_dtypes.bfloat16), degl


_GSLOT = [(0, 0), (32, 0), (64, 0), (32, 512)]
_CACHE = {}
TRACE = False


def prepare_in_maps(inputs):
    f32 = lambda x: np.ascontiguousarray(np.asarray(x, np.float32))
    bf16 = lambda x: np.ascontiguousarray(
        np.asarray(x, np.float32).astype(ml_dtypes.bfloat16))
    feat = f32(inputs["feat"])
    edge_src = np.asarray(inputs["edge_src"])
    edge_dst = np.asarray(inputs["edge_dst"])
    W = {k: tf32_round(inputs[k]) for k in
         ("W1", "W2", "W3", "aW1", "mW1", "mW2")}
    Wb = {k: bf16(inputs[k]) for k in
          ("aW2", "aW3", "pW", "qW1", "qW2", "qW3")}
    b = {k: f32(inputs[k]) for k in
         ("b1", "b2", "b3", "ab1", "ab2", "ab3", "pb", "qb1", "qb2", "qb3",
          "mb1", "mb2")}
    ident = np.eye(128, dtype=np.float32)
    rows2 = _pack_rows(b)

    in_maps = []
    for c in range(NCORES):
        fs = feat[c * NLOC : (c + 1) * NLOC]
        feat_nm = np.ascontiguousarray(
            fs.reshape(T, 128, IN).transpose(1, 0, 2).reshape(128, T * IN))
        featT = np.ascontiguousarray(fs.T)
        at_idx, at_val, degl = _edge_tables(edge_src, edge_dst, c)
        pW_lc = np.ascontiguousarray(Wb["pW"][:, c * G * K : (c + 1) * G * K])
        pb_lc = np.ascontiguousarray(b["pb"][c * G * K : (c + 1) * G * K])
        degr = np.zeros((65, 1024), np.float32)
        for g in range(G):
            p, off = _GSLOT[g]
            degr[p, off : off + NPG] = degl[g * NPG : (g + 1) * NPG]
        in_maps.append({
            "featT": tf32_round(featT),
            "feat_nm": tf32_round(feat_nm),
            "at_idx": at_idx, "at_val": at_val,
            "bcol": _pack_bcol(b, pb_lc), "rows2": rows2,
            "ident": ident, "identr": ident,
            "degr": degr,
            "W1": W["W1"], "W2": W["W2"], "W3": W["W3"],
            "aW1": W["aW1"], "aW2": Wb["aW2"], "aW3": Wb["aW3"],
            "pWl": pW_lc, "qW1": Wb["qW1"], "qW2": Wb["qW2"], "qW3": Wb["qW3"],
            "mW1": W["mW1"], "mW2": W["mW2"],
        })
    return in_maps


def kernel(**inputs):
    if "nc" not in _CACHE:
        _CACHE["nc"] = build_module()
    nc = _CACHE["nc"]
    in_maps = prepare_in_maps(inputs)
    res = run_bass_kernel_spmd(nc, in_maps, core_ids=list(range(NCORES)),
                               trace=TRACE)
    _CACHE["last_res"] = res
    out = np.zeros((B, 10), np.float32)
    for c in range(NCORES):
        out[c * G : (c + 1) * G, :] = np.asarray(res.results[c]["yp"]).T
    return out


# revision 59
# speedup vs baseline: 1.7387x; 1.3319x over previous
"""DiffPoolEncoder Trainium2 kernel.

Sharding: data parallel by graph. 8 cores x 4 graphs (512 nodes each).
Per core the whole network runs on-device; GraphSage aggregation uses dense
per-graph A^T tiles built on-device via gpsimd local_scatter from
host-prepared (dst, count) index tables (index-only preprocessing of the
edge list; clamped in-degrees are integer edge counts and ship as exact
f32). The h-branch (readout) runs in float32r; the assignment branch,
pooled stage and all spill-free node-major copies run in bfloat16 --
matmul row rate is identical, but bf16 halves SBUF/DMA and avoids the
f32r 4-cycles-per-row penalty on narrow outputs. Softmax, A@S, S^T X and
S^T A S are fused per-graph into the a3/logits loop so the pooled tail
overlaps the big assignment matmuls. SBUF pools are strictly LIFO-nested
by lifetime epoch.
"""

import sys

for _p in ("/opt/trn_rl_repo",):
    if _p not in sys.path:
        sys.path.append(_p)

import numpy as np
import ml_dtypes
from contextlib import ExitStack

import concourse.bass as bass
import concourse.mybir as mybir
import concourse.tile as tile
from concourse import bacc
from concourse.bass_utils import run_bass_kernel_spmd

F32 = mybir.dt.float32
F32R = mybir.dt.float32r
BF16 = mybir.dt.bfloat16
FP8 = mybir.dt.float8e4
I16 = mybir.dt.int16
AF = mybir.ActivationFunctionType
ALU = mybir.AluOpType
AX = mybir.AxisListType

NCORES = 8
B = 32
NPG = 512
G = 4            # graphs per core
T = 16           # node tiles per core (4 per graph)
NLOC = 2048      # nodes per core
K = 64           # clusters per graph
IN = 128
HID = 256
NI = 48          # padded (dst,count) entries per (src-tile, partition)

# bcol column layout (each 128-chunk of a bias vector is one column)
BC_B1, BC_B2, BC_B3 = 0, 2, 4
BC_AB1, BC_AB2 = 6, 8
BC_AB3 = 10          # 16 cols
BC_QB1, BC_QB2, BC_QB3 = 26, 28, 30
BC_MB1, BC_MB2 = 32, 34
BC_PB = 35           # 4 cols: per-graph local pb, partitions 0:64
BC_N = 39

# rows2 [65, 1536] f32r: rows at matmul base partitions {0, 32, 64};
# ones[0:512] replicated at each used partition (matmul needs equal bases).
R_QB1 = (0, 512)
R_QB2, R_QB3 = (64, 512), (64, 768)
ROWS_W = 1024


def build_module():
    nc = bacc.Bacc("TRN2", target_bir_lowering=False)

    # ---------------- DRAM I/O ----------------
    featT_d = nc.dram_tensor("featT", [128, NLOC], F32R, kind="ExternalInput")
    featnm_d = nc.dram_tensor("feat_nm", [128, T * IN], F32R, kind="ExternalInput")
    atidx_d = nc.dram_tensor("at_idx", [128, T * NI], I16, kind="ExternalInput")
    atval_d = nc.dram_tensor("at_val", [128, T * NI], BF16, kind="ExternalInput")
    bcol_d = nc.dram_tensor("bcol", [128, BC_N], F32, kind="ExternalInput")
    rows_d = nc.dram_tensor("rows2", [65, ROWS_W], F32R, kind="ExternalInput")
    ident_d = nc.dram_tensor("ident", [128, 128], F32, kind="ExternalInput")
    identr_d = nc.dram_tensor("identr", [128, 128], F32R, kind="ExternalInput")
    # clamped in-degree (exact integer counts), per-graph gslot rows
    degr_d = nc.dram_tensor("degr", [65, 1024], F32R, kind="ExternalInput")
    w_d = {}
    for name, fi, fo, dt in [
        ("W1", 256, 256, F32R), ("W2", 512, 256, F32R), ("W3", 512, 256, F32R),
        ("aW1", 256, 256, F32R), ("aW2", 512, 256, BF16),
        ("aW3", 512, 2048, mybir.dt.float8e4),
        ("pW12", 512, 256, BF16), ("pW3", 2048, 256, mybir.dt.float8e4),
        ("qW1", 1536, 256, BF16), ("qW2", 512, 256, BF16),
        ("qW3", 512, 256, BF16), ("mW1", 1536, 256, F32R),
        ("mW2", 256, 10, F32R),
    ]:
        w_d[name] = nc.dram_tensor(name, [fi, fo], dt, kind="ExternalInput")
    yp_d = nc.dram_tensor("yp", [10, G], F32, kind="ExternalOutput")

    with tile.TileContext(nc) as tc, ExitStack() as ex, \
            nc.allow_low_precision(reason="f32r/bf16 matmuls; accumulation stays fp32 in PSUM"):
        persist = ex.enter_context(tc.tile_pool(name="persist", bufs=1))
        # PSUM: 8 banks. One tag per pool so slot count == bank count.
        ps_p = ex.enter_context(tc.tile_pool(name="psP", bufs=3, space="PSUM"))
        lg_p = ex.enter_context(tc.tile_pool(name="psL", bufs=1, space="PSUM"))
        pm_p = ex.enter_context(tc.tile_pool(name="psM", bufs=2, space="PSUM"))
        pl_p = ex.enter_context(tc.tile_pool(name="psS", bufs=2, space="PSUM"))

        uid = [0]

        def _nm(pfx):
            uid[0] += 1
            return f"{pfx}{uid[0]}"

        def ps_big(dt=F32):
            return ps_p.tile([128, 512], dt, tag="ps", name=_nm("ps"))

        def ps_med(p, f, dt=F32):
            return pm_p.tile([p, f], dt, tag="pm", name=_nm("pm"))

        def ps_sml(p, f, dt=F32):
            return pl_p.tile([p, f], dt, tag="pl", name=_nm("pl"))

        def wload(pool, name, fi, fo, dt=F32R):
            kk = fi // 128
            sb = pool.tile([128, kk * fo], dt, tag=name, name=name)
            nc.sync.dma_start(
                sb[:].rearrange("p (k f) -> p k f", k=kk, f=fo),
                w_d[name][:, :].rearrange("(k p) f -> p k f", p=128),
            )
            return sb

        # ---------- persistent small tensors (epoch E6) ----------
        ident = persist.tile([128, 128], F32)
        identr = persist.tile([128, 128], F32R)
        identb = persist.tile([128, 128], BF16)
        bcol = persist.tile([128, BC_N], F32)
        ones_pr = persist.tile([65, 128], F32R)  # ones rows at base partitions
        nmax = persist.tile([128, 2], F32)
        sumx = persist.tile([128, 2], F32)
        nc.vector.memset(ones_pr[:].bitcast(F32), 1.0)

        def ones_at(p, n):
            return ones_pr[p : p + 1, 0:n]

        # right-side stack: tensors produced mid-stream and consumed by the
        # pooled tail; outlives the left-side phase pools.
        keep = ex.enter_context(tc.tile_pool(name="keep", bufs=1, side="right"))
        out_fm = keep.tile([128, 12 * G], F32R)  # readout maxes, col=ch*G+g
        y_sb = keep.tile([128, 2 * G], F32R)
        z_sb = keep.tile([10, G], F32)
        # node-major g_emb layers, bf16, for the pooled stage (no DRAM spill)
        h1nb = keep.tile([128, T * HID], BF16)
        h2nb = keep.tile([128, T * HID], BF16)

        # AT (scaled A^T tiles, f32r) lives until AS (epoch E6)
        at_p = ex.enter_context(tc.tile_pool(name="atp", bufs=1))
        AT = at_p.tile([128, T * NPG], F32R)

        # ---------- LIFO phase pools ----------
        ex5 = ExitStack()   # close after logits/softmax
        agg_p = ex5.enter_context(tc.tile_pool(name="aggfm", bufs=1))
        afm1_p = ex5.enter_context(tc.tile_pool(name="afm1", bufs=1))
        afm2_p = ex5.enter_context(tc.tile_pool(name="afm2", bufs=1))
        ex3 = ExitStack()   # close after h3 (h3 runs after xnm closes)
        xfm_p = ex3.enter_context(tc.tile_pool(name="xfm", bufs=2))
        w3_p = ex3.enter_context(tc.tile_pool(name="w3p", bufs=1))
        ex4 = ExitStack()   # close after agg_a2
        xnm_p = ex4.enter_context(tc.tile_pool(name="xnm", bufs=2))
        ex2c = ExitStack()  # close after a2
        a2w_p = ex2c.enter_context(tc.tile_pool(name="a2wp", bufs=1))
        ex2b = ExitStack()  # close after h2
        w2_p = ex2b.enter_context(tc.tile_pool(name="w2p", bufs=1))
        ex2 = ExitStack()   # close after h1/a1
        w1_p = ex2.enter_context(tc.tile_pool(name="w1p", bufs=1))
        agf_p = ex2.enter_context(tc.tile_pool(name="agfp", bufs=1))
        ex1 = ExitStack()   # close after agg_feat
        fnm_p = ex1.enter_context(tc.tile_pool(name="fnmp", bufs=1))
        ex0 = ExitStack()   # close after A^T built+scaled
        ate_p = ex0.enter_context(tc.tile_pool(name="atep", bufs=1))

        # ---------- input DMAs ----------
        atbf_idx = ate_p.tile([128, T * NI], I16, tag="atidx")
        atbf_val = ate_p.tile([128, T * NI], BF16, tag="atval")
        # DMA priority order: the scatter/scale chain first, then layer-1
        # activations, then constants and weights.
        dinvr = ate_p.tile([65, 1024], F32R, tag="dinvr")  # deg -> 1/deg rows
        nc.sync.dma_start(dinvr[:], degr_d[:])
        nc.sync.dma_start(atbf_idx[:], atidx_d[:])
        nc.sync.dma_start(atbf_val[:], atval_d[:])
        featnm = fnm_p.tile([128, T * IN], F32R, tag="featnm")
        nc.sync.dma_start(featnm[:], featnm_d[:])
        ftg = []
        for g in range(G):
            ft = fnm_p.tile([128, NPG], F32R, tag="ftc", name=_nm("ftc"),
                            bufs=2)
            nc.sync.dma_start(ft[:], featT_d[:, g * NPG : (g + 1) * NPG])
            ftg.append(ft)
        nc.sync.dma_start(ident[:], ident_d[:])
        nc.sync.dma_start(identr[:], identr_d[:])
        nc.sync.dma_start(bcol[:], bcol_d[:])
        W1 = wload(w1_p, "W1", 256, 256)
        aW1 = wload(w1_p, "aW1", 256, 256)
        W2 = wload(w2_p, "W2", 512, 256)
        W3 = wload(w3_p, "W3", 512, 256)
        aW2 = wload(a2w_p, "aW2", 512, 256, BF16)

        gslot = [(0, 0), (32, 0), (64, 0), (32, 512)]
        for g in range(G):
            p, off = gslot[g]
            nc.vector.reciprocal(dinvr[p : p + 1, off : off + NPG],
                                 dinvr[p : p + 1, off : off + NPG])

        # ---------- emit helpers ----------
        def emit_agg(x_nm, D, out_t, glist=range(G)):
            """out_t[d, n] (feature-major) = sum_s x_nm[s, d] * AT[s, n]."""
            for g in glist:
                for ch in range(D // 128):
                    ps = ps_big()
                    for st in range(4):
                        t = g * 4 + st
                        nc.tensor.matmul(
                            ps[:],
                            lhsT=x_nm[:, t * D + ch * 128 : t * D + ch * 128 + 128],
                            rhs=AT[:, t * NPG : (t + 1) * NPG],
                            start=(st == 0), stop=(st == 3))
                    # AT holds raw edge counts; the mean-normalization by
                    # 1/deg[dst] rides along with the DVE evac for free
                    nc.vector.tensor_tensor(
                        out=out_t[:, ch * NLOC + g * NPG : ch * NLOC + (g + 1) * NPG],
                        in0=ps[:], in1=dbc[:, g * NPG : (g + 1) * NPG],
                        op=ALU.mult)

        def emit_lin_fm(x_fm, a_fm, Din, Dout, Wsb, bccol, relu, out_t,
                        nblist=range(4)):
            nk = Din // 128
            for co in range(Dout // 128):
                for nb in nblist:
                    ps = ps_big()
                    ki = 0
                    for src in (x_fm, a_fm):
                        for ci in range(nk):
                            nc.tensor.matmul(
                                ps[:],
                                lhsT=Wsb[:, ki * Dout + co * 128 : ki * Dout + co * 128 + 128],
                                rhs=src[:, ci * NLOC + nb * 512 : ci * NLOC + (nb + 1) * 512],
                                start=(ki == 0), stop=(ki == 2 * nk - 1))
                            ki += 1
                    nc.scalar.activation(
                        out_t[:, co * NLOC + nb * 512 : co * NLOC + (nb + 1) * 512],
                        ps[:], AF.Relu if relu else AF.Identity,
                        bias=bcol[:, bccol + co : bccol + co + 1])

        def emit_nm_T(x_fm, out_nm, dt=F32R, bcopy=None, t2list=None):
            # node-major via PE transposes of the (already relu'd) fm tensor;
            # 4 transposed blocks share one PSUM bank -> single 512-wide evac.
            # bcopy: persistent bf16 copy evacuated on the idle Pool engine.
            idm = identb if dt == BF16 else identr
            for t2 in (range(0, T, 2) if t2list is None else t2list):
                tp = ps_big(dt)
                for i, (t, ch) in enumerate(
                        ((t2, 0), (t2, 1), (t2 + 1, 0), (t2 + 1, 1))):
                    nc.tensor.matmul(
                        tp[:, i * 128 : (i + 1) * 128],
                        lhsT=x_fm[:, ch * NLOC + t * 128 : (ch * NLOC + t * 128) + 128],
                        rhs=idm[:], is_transpose=True,
                        start=True, stop=True, skip_group_check=True)
                sl = slice(t2 * HID, (t2 + 2) * HID)
                if out_nm is not None:
                    if dt == BF16:
                        nc.scalar.copy(out_nm[:, sl], tp[:])
                    else:
                        nc.vector.tensor_copy(out_nm[:, sl], tp[:])
                    if bcopy is not None:
                        # bf16 shadow for the pooled tail; SBUF->SBUF on the
                        # idle Pool engine (GPSIMD cannot read PSUM)
                        nc.gpsimd.tensor_copy(bcopy[:, sl], out_nm[:, sl])
                elif bcopy is not None:
                    nc.scalar.copy(bcopy[:, sl], tp[:])

        def emit_out1(x_fm, ch0, glist=range(G)):
            for ci in range(2):
                for g in glist:
                    nc.vector.tensor_reduce(
                        out_fm[:, (ch0 + ci) * G + g : (ch0 + ci) * G + g + 1],
                        x_fm[:, ci * NLOC + g * NPG : ci * NLOC + (g + 1) * NPG],
                        axis=AX.X, op=ALU.max)

        # ---------- phase 0 fused with h1/a1, pipelined per graph ----------
        aggfeat = agf_p.tile([128, NLOC], F32R, tag="aggf", name="aggfeat")
        h1f = xfm_p.tile([128, 2 * NLOC], F32R, tag="xfm", name="h1f")
        h1n = xnm_p.tile([128, T * HID], F32R, tag="xnm", name="h1n")
        a1f = afm1_p.tile([128, 2 * NLOC], BF16, tag="a1f", name="a1f")
        a1n = xnm_p.tile([128, T * HID], F32R, tag="xnm", name="a1n")

        def emit_lin1(ftc, g, Wsb, bccol, out_t):
            # layer-1 linear for one graph: cat(feat, agg_feat) @ W
            for co in range(2):
                ps = ps_big()
                nc.tensor.matmul(
                    ps[:], lhsT=Wsb[:, co * 128 : co * 128 + 128],
                    rhs=ftc[:], start=True, stop=False)
                nc.tensor.matmul(
                    ps[:], lhsT=Wsb[:, 256 + co * 128 : 256 + co * 128 + 128],
                    rhs=aggfeat[:, g * NPG : (g + 1) * NPG],
                    start=False, stop=True)
                nc.scalar.activation(
                    out_t[:, co * NLOC + g * NPG : co * NLOC + (g + 1) * NPG],
                    ps[:], AF.Relu,
                    bias=bcol[:, bccol + co : bccol + co + 1])

        # A^T build first for ALL graphs: keeps the in-order Pool queue
        # (scatters) free of later-phase ops, so graph g+1's scatter never
        # waits behind graph g's h1 consumers. AT keeps RAW counts; the
        # 1/deg broadcast tiles (dbc) are applied at every agg evac instead.
        dbc = agg_p.tile([128, G * NPG], F32R, tag="dbc", name="dbc")
        for g in range(G):
            p, off = gslot[g]
            bps = ps_big()
            nc.tensor.matmul(bps[:], lhsT=ones_at(p, 128),
                             rhs=dinvr[p : p + 1, off : off + NPG],
                             start=True, stop=True)
            nc.scalar.copy(dbc[:, g * NPG : (g + 1) * NPG], bps[:])
        for t in range(T):
            scr = ate_p.tile([128, NPG], BF16, tag="scscr", name=_nm("sc"),
                             bufs=2)
            nc.gpsimd.local_scatter(
                out_ap=scr[:],
                data_ap=atbf_val[:, t * NI : (t + 1) * NI],
                idxs_ap=atbf_idx[:, t * NI : (t + 1) * NI],
                channels=128, num_elems=NPG, num_idxs=NI,
            )
            # raw A^T tile; alternate evac engine to balance Act/DVE
            if t % 2 == 0:
                nc.scalar.copy(AT[:, t * NPG : (t + 1) * NPG], scr[:])
            else:
                nc.vector.tensor_copy(AT[:, t * NPG : (t + 1) * NPG], scr[:])
        nc.vector.tensor_copy(identb[:], ident[:])
        for g in range(G):
            emit_agg(featnm, IN, aggfeat, glist=[g])
            emit_lin1(ftg[g], g, W1, BC_B1, h1f)
            emit_lin1(ftg[g], g, aW1, BC_AB1, a1f)
            emit_nm_T(h1f, h1n, bcopy=h1nb, t2list=(4 * g, 4 * g + 2))
            emit_nm_T(a1f, a1n, dt=BF16, t2list=(4 * g, 4 * g + 2))
        emit_out1(h1f, 0)
        ex0.close()
        ex1.close()
        ex2.close()

        aggh1 = agg_p.tile([128, 2 * NLOC], F32R, tag="aggh", name="aggh1")
        emit_agg(h1n, HID, aggh1)

        agga1 = agg_p.tile([128, 2 * NLOC], BF16, tag="agga", name="agga1")
        emit_agg(a1n, HID, agga1)

        h2f = xfm_p.tile([128, 2 * NLOC], F32R, tag="xfm", name="h2f")
        h2n = xnm_p.tile([128, T * HID], F32R, tag="xnm", name="h2n")
        emit_lin_fm(h1f, aggh1, 256, 256, W2, BC_B2, True, h2f)
        emit_nm_T(h2f, h2n, bcopy=h2nb)
        emit_out1(h2f, 2)
        ex2b.close()

        a2f = afm2_p.tile([128, 2 * NLOC], BF16, tag="a2f", name="a2f")
        a2n = xnm_p.tile([128, T * HID], F32R, tag="xnm", name="a2n")
        emit_lin_fm(a1f, agga1, 256, 256, aW2, BC_AB2, True, a2f)
        emit_nm_T(a2f, a2n, dt=BF16)
        ex2c.close()

        aggh2 = agg_p.tile([128, 2 * NLOC], F32R, tag="aggh", name="aggh2")
        emit_agg(h2n, HID, aggh2)

        agga2 = agg_p.tile([128, 2 * NLOC], BF16, tag="agga", name="agga2")
        emit_agg(a2n, HID, agga2)
        ex4.close()

        # pooled-stage tiles + weights: prefetch during the a3/logits phase.
        # Right-side SBUF stack: outlives the left-side phase pools without
        # violating their LIFO discipline.
        late = ex.enter_context(tc.tile_pool(name="late", bufs=1, side="right"))
        rows2 = late.tile([65, ROWS_W], F32R, tag="rows2", name="rows2")
        nc.sync.dma_start(rows2[:], rows_d[:])

        def rrow(ro, n):
            p, off = ro
            return rows2[p : p + 1, off : off + n]

        qW1 = wload(late, "qW1", 1536, 256, BF16)
        # fp8 shadows of the a3 operands (DVE converting copies, emitted
        # here so they overlap h3 instead of stalling the a3 loop entry)
        a2f8 = late.tile([128, 2 * NLOC], FP8, tag="a2f8", name="a2f8")
        agga28 = late.tile([128, 2 * NLOC], FP8, tag="agga28", name="agga28")
        for ci in range(2):
            nc.vector.tensor_copy(a2f8[:, ci * NLOC : (ci + 1) * NLOC],
                                  a2f[:, ci * NLOC : (ci + 1) * NLOC])
            nc.vector.tensor_copy(agga28[:, ci * NLOC : (ci + 1) * NLOC],
                                  agga2[:, ci * NLOC : (ci + 1) * NLOC])
        mW1 = wload(late, "mW1", 1536, 256)
        mW2 = wload(late, "mW2", 256, 10)
        qW2 = wload(late, "qW2", 512, 256, BF16)
        qW3 = wload(late, "qW3", 512, 256, BF16)
        S_b = late.tile([128, T * K], BF16, tag="S_b", name="S_b")
        h3nb = late.tile([128, T * HID], BF16, tag="h3nb", name="h3nb")
        AS_nm = late.tile([128, T * K], BF16, tag="AS", name="AS_nm")
        hp_nm = late.tile([128, 2 * 768], BF16, tag="hpn", name="hp_nm")
        hp_fm = late.tile([128, 6 * 256], BF16, tag="hpf", name="hp_fm")
        adjg = late.tile([128, 2 * K], F32, tag="adjg", name="adjg")
        rsum = late.tile([128, 2], F32, tag="rsum", name="rsum")
        adjT = late.tile([128, 2 * 128], BF16, tag="adjT", name="adjT")
        hn1_fm = late.tile([128, 6 * 256], BF16, tag="hn1", name="hn1_fm")
        p1_nm = late.tile([128, 2 * 256], BF16, tag="p1n", name="p1_nm")
        p1_fm = late.tile([128, 2 * 256], BF16, tag="p1f", name="p1_fm")
        hn2_fm = late.tile([128, 2 * 256], BF16, tag="hn2", name="hn2_fm")
        p2_nm = late.tile([128, 2 * 256], BF16, tag="p2n", name="p2_nm")
        p2_fm = late.tile([128, 2 * 256], BF16, tag="p2f", name="p2_fm")
        hn3_fm = late.tile([128, 2 * 256], BF16, tag="hn3", name="hn3_fm")
        p3_fm = late.tile([128, 2 * 256], BF16, tag="p3f", name="p3_fm")
        nc.vector.memset(adjT[:].bitcast(F32), 0.0)
        Xr = [h1nb, h2nb, h3nb]

        # h3: fm + readout; node-major -> persistent bf16 only
        h3f = xfm_p.tile([128, 2 * NLOC], F32R, tag="xfm", name="h3f")
        emit_lin_fm(h2f, aggh2, 256, 256, W3, BC_B3, False, h3f)
        emit_nm_T(h3f, None, bcopy=h3nb)
        emit_out1(h3f, 4)
        ex3.close()

        # a3/logits weights on the right stack, opened once xfm/w3 are gone
        ex5b = ExitStack()
        wl_p = ex5b.enter_context(tc.tile_pool(name="wlate", bufs=1,
                                               side="right"))
        aW3 = wl_p.tile([128, 4 * 2048], FP8, tag="aW3", name="aW3")
        aW3v = aW3[:].rearrange("p (k f) -> p k f", k=4, f=2048)
        for q in range(4):
            nc.sync.dma_start(
                aW3v[:, :, q * 512 : (q + 1) * 512],
                w_d["aW3"][:, q * 512 : (q + 1) * 512].rearrange(
                    "(k p) f -> p k f", p=128))
        pW12 = wload(wl_p, "pW12", 512, 256, BF16)
        pW3 = wload(wl_p, "pW3", 2048, 256, mybir.dt.float8e4)

        # ---------- a3 + logits + softmax + AS + pooled prep, per graph ----
        def emit_hpool(h):
            # both graphs of the pair into one PSUM tile via partition-offset
            # outputs (base partition 64 for the odd graph) -> single evac
            for L in range(3):
                ps = ps_med(128, 256)
                for gs in range(2):
                    g = h * 2 + gs
                    for j in range(4):
                        t = g * 4 + j
                        nc.tensor.matmul(
                            ps[gs * 64 : gs * 64 + 64, :],
                            lhsT=S_b[:, t * K : (t + 1) * K],
                            rhs=Xr[L][:, t * HID : (t + 1) * HID],
                            start=(j == 0), stop=(j == 3),
                            skip_group_check=True)
                nc.vector.tensor_copy(
                    hp_nm[:, h * 768 + L * 256 : h * 768 + (L + 1) * 256],
                    ps[:])
            for ch in range(6):  # hp_fm via transposes of the pair tile
                tp = ps_med(128, 128, BF16)
                nc.tensor.matmul(
                    tp[:], lhsT=hp_nm[:, h * 768 + ch * 128 : h * 768 + (ch + 1) * 128],
                    rhs=identb[:], is_transpose=True,
                    start=True, stop=True, skip_group_check=True)
                nc.vector.tensor_copy(
                    hp_fm[:, ch * 256 + h * 128 : ch * 256 + (h + 1) * 128], tp[:])

        def emit_adj(h):
            ps = ps_sml(128, K)
            for gs in range(2):
                g = h * 2 + gs
                for j in range(4):
                    t = g * 4 + j
                    nc.tensor.matmul(ps[gs * 64 : gs * 64 + 64, :],
                                     lhsT=S_b[:, t * K : (t + 1) * K],
                                     rhs=AS_nm[:, t * K : (t + 1) * K],
                                     start=(j == 0), stop=(j == 3),
                                     skip_group_check=True)
            nc.vector.tensor_copy(adjg[:, h * K : (h + 1) * K], ps[:])
            nc.vector.tensor_reduce(rsum[:, h : h + 1], adjg[:, h * K : (h + 1) * K],
                                    axis=AX.X, op=ALU.add)
            nc.vector.tensor_scalar(rsum[:, h : h + 1], rsum[:, h : h + 1],
                                    1e-9, None, op0=ALU.add)
            nc.vector.reciprocal(rsum[:, h : h + 1], rsum[:, h : h + 1])
            nc.vector.tensor_scalar(adjg[:, h * K : (h + 1) * K],
                                    adjg[:, h * K : (h + 1) * K],
                                    rsum[:, h : h + 1], None, op0=ALU.mult)
            # transpose each graph's [64,64] block onto the block diagonal
            # (transpose outputs must land at PSUM partition 0; odd block is
            # partition-shifted into place with a small SBUF->SBUF DMA)
            for gs in range(2):
                tp = ps_sml(128, K)
                nc.tensor.transpose(
                    tp[0:64, :],
                    adjg[gs * 64 : gs * 64 + 64, h * K : (h + 1) * K],
                    ident[gs * 64 : gs * 64 + 64, gs * 64 : gs * 64 + 64]
                    if gs else ident[0:64, 0:64])
                if gs == 0:
                    nc.vector.tensor_copy(adjT[0:64, h * 128 : h * 128 + 64],
                                          tp[0:64, :])
                else:
                    sb = late.tile([64, K], BF16, tag="adjsh", name=_nm("adjsh"),
                                   bufs=2)
                    nc.vector.tensor_copy(sb[:], tp[0:64, :])
                    nc.sync.dma_start(
                        adjT[64:128, h * 128 + 64 : h * 128 + 128], sb[:])

        def pool_hn_h(x_nm, xw, out_t, h):
            # out[d, u] = sum_v x_nm[v, d] * adjT_bd[v, u], one graph pair
            for ch in range(xw // 128):
                tp = ps_sml(128, 128)
                nc.tensor.matmul(
                    tp[:],
                    lhsT=x_nm[:, h * xw + ch * 128 : h * xw + (ch + 1) * 128],
                    rhs=adjT[:, h * 128 : (h + 1) * 128],
                    start=True, stop=True)
                nc.vector.tensor_copy(
                    out_t[:, ch * 256 + h * 128 : ch * 256 + (h + 1) * 128],
                    tp[:])

        def pool_lin_h(xf, hf, Din, Wsb, bccol, rbias, relu, ch0, outf, outn,
                       h):
            nch = Din // 256
            for co in range(2):
                ps = ps_med(128, 128)
                ki = 0
                for src in (xf, hf):
                    for ch in range(nch):
                        nc.tensor.matmul(
                            ps[:],
                            lhsT=Wsb[:, ki * 256 + co * 128 : ki * 256 + co * 128 + 128],
                            rhs=src[:, ch * 256 + h * 128 : ch * 256 + (h + 1) * 128],
                            start=(ki == 0), stop=(ki == 2 * nch - 1))
                        ki += 1
                nc.scalar.activation(
                    outf[:, co * 256 + h * 128 : co * 256 + (h + 1) * 128],
                    ps[:], AF.Relu if relu else AF.Identity,
                    bias=bcol[:, bccol + co : bccol + co + 1])
                for gs in range(2):
                    g2 = 2 * h + gs
                    nc.vector.tensor_reduce(
                        out_fm[:, (ch0 + co) * G + g2 : (ch0 + co) * G + g2 + 1],
                        outf[:, co * 256 + h * 128 + gs * K : co * 256 + h * 128 + (gs + 1) * K],
                        axis=AX.X, op=ALU.max)
            if outn is not None:
                ps = ps_med(128, 256)
                ki = 0
                for src in (xf, hf):
                    for ch in range(nch):
                        nc.tensor.matmul(
                            ps[:],
                            lhsT=src[:, ch * 256 + h * 128 : ch * 256 + (h + 1) * 128],
                            rhs=Wsb[:, ki * 256 : (ki + 1) * 256],
                            start=(ki == 0), stop=False)
                        ki += 1
                nc.tensor.matmul(ps[:], lhsT=ones_at(rbias[0], 128).bitcast(F32),
                                 rhs=rrow(rbias, 256).bitcast(F32),
                                 start=False, stop=True)
                nc.vector.tensor_scalar(outn[:, h * 256 : (h + 1) * 256], ps[:],
                                        0.0, None, op0=ALU.max)

        for g in range(G):
            lps = lg_p.tile([64, 512], F32, tag="lg", name=_nm("lg"))
            a3p = None
            for co in range(16):  # a3 = relu(cat(a2, agg_a2) @ aW3 + ab3)
                # fp8e4m3 DoubleRow: two 128-deep k-tiles per matmul at
                # 0.5 cycles/row; weights pre-scaled x64 on host, undone by
                # the evac's activation scale. The relu'd a3 is kept in fp8
                # pair-tiles so the logits a3-block also runs DoubleRow
                # (pW3 is x64 fp8; the lgf evac divides the sum back down).
                ps3 = ps_big()
                aw = aW3[:].rearrange("p (k f) -> p k f", k=4, f=2048)
                for q, src8 in enumerate((a2f8, agga28)):
                    nc.tensor.matmul(
                        ps3[:],
                        lhsT=aw[:, 2 * q : 2 * q + 2,
                                co * 128 : co * 128 + 128],
                        rhs=src8[:].rearrange("p (k n) -> p k n", k=2,
                                              n=NLOC)[:, :, g * NPG : (g + 1) * NPG],
                        start=(q == 0), stop=(q == 1),
                        perf_mode=mybir.MatmulPerfMode.DoubleRow)
                if co % 2 == 0:
                    a3p = wl_p.tile([128, 2 * 512], mybir.dt.float8e4,
                                    tag="a3p", name=_nm("a3p"), bufs=2)
                dst8 = a3p[:, (co % 2) * 512 : (co % 2 + 1) * 512]
                if co % 2 == 0:
                    nc.scalar.activation(
                        dst8, ps3[:], AF.Relu,
                        bias=bcol[:, BC_AB3 + co : BC_AB3 + co + 1])
                else:
                    nc.vector.tensor_scalar(
                        dst8, ps3[:],
                        bcol[:, BC_AB3 + co : BC_AB3 + co + 1], 0.0,
                        op0=ALU.add, op1=ALU.max)
                if co % 2 == 1:
                    qq = co // 2
                    nc.tensor.matmul(
                        lps[:],
                        lhsT=pW3[:].rearrange("p (k f) -> p k f", k=16,
                                              f=256)[:, 2 * qq : 2 * qq + 2,
                                                     g * K : g * K + K],
                        rhs=a3p[:].rearrange("p (k n) -> p k n", k=2, n=512),
                        start=(qq == 0), stop=False,
                        perf_mode=mybir.MatmulPerfMode.DoubleRow)
            for ci in range(2):  # a1 block of pW
                nc.tensor.matmul(
                    lps[:], lhsT=pW12[:, ci * 256 + g * K : ci * 256 + g * K + K],
                    rhs=a1f[:, ci * NLOC + g * NPG : ci * NLOC + (g + 1) * NPG],
                    start=False, stop=False)
            for ci in range(2):  # a2 block
                nc.tensor.matmul(
                    lps[:], lhsT=pW12[:, (2 + ci) * 256 + g * K : (2 + ci) * 256 + g * K + K],
                    rhs=a2f[:, ci * NLOC + g * NPG : ci * NLOC + (g + 1) * NPG],
                    start=False, stop=(ci == 1))
            lgf = wl_p.tile([64, 512], F32, tag="lgf", name=_nm("lgf"), bufs=2)
            nc.scalar.activation(lgf[:], lps[:], AF.Identity, scale=1.0 / 1024,
                                 bias=bcol[0:64, BC_PB + g : BC_PB + g + 1])
            lgt = wl_p.tile([128, 4 * K], F32, tag="lgs", name=_nm("lgs"),
                            bufs=2)
            for j in range(4):  # transpose to node-major
                tps = ps_med(128, 64)
                nc.tensor.transpose(tps[:], lgf[0:64, j * 128 : (j + 1) * 128],
                                    ident[0:64, 0:64])
                nc.vector.tensor_copy(lgt[:, j * K : (j + 1) * K], tps[:])

            # masked softmax == per-graph softmax over K columns. S kept in
            # two forms: transient f32r (for the f32r AS matmuls against AT)
            # and persistent bf16 (for the bf16 pooled stage) — the compiler
            # rejects 32-bit x 16-bit matmul operand mixes.
            S32 = wl_p.tile([128, 4 * K], F32R, tag="S32", name=_nm("S32"),
                            bufs=2)
            for j in range(4):
                t = g * 4 + j
                bb = t % 2
                nc.vector.tensor_reduce(nmax[:, bb : bb + 1],
                                        lgt[:, j * K : (j + 1) * K],
                                        axis=AX.X, op=ALU.max, negate=True)
                nc.scalar.activation(S32[:, j * K : (j + 1) * K],
                                     lgt[:, j * K : (j + 1) * K], AF.Exp,
                                     bias=nmax[:, bb : bb + 1],
                                     accum_out=sumx[:, bb : bb + 1])
                nc.vector.reciprocal(sumx[:, bb : bb + 1], sumx[:, bb : bb + 1])
                nc.vector.tensor_scalar(S32[:, j * K : (j + 1) * K],
                                        S32[:, j * K : (j + 1) * K],
                                        sumx[:, bb : bb + 1], None, op0=ALU.mult)
                nc.gpsimd.tensor_copy(S_b[:, t * K : (t + 1) * K],
                                      S32[:, j * K : (j + 1) * K])

            # AS = A @ S: scaled-AT product un-scaled by clamped deg (exact)
            for j in range(4):
                t = g * 4 + j
                ps = ps_sml(128, K)
                for st in range(4):
                    nc.tensor.matmul(
                        ps[:],
                        lhsT=AT[:, (g * 4 + st) * NPG + j * 128 : (g * 4 + st) * NPG + (j + 1) * 128],
                        rhs=S32[:, st * K : (st + 1) * K],
                        start=(st == 0), stop=(st == 3))
                nc.vector.tensor_copy(AS_nm[:, t * K : (t + 1) * K], ps[:])
            if g % 2 == 1:  # pooled stack for the completed pair
                h = g // 2
                emit_hpool(h)
                emit_adj(h)
                pool_hn_h(hp_nm, 768, hn1_fm, h)
                pool_lin_h(hp_fm, hn1_fm, 1536, qW1, BC_QB1, R_QB1, True, 6,
                           p1_fm, p1_nm, h)
                pool_hn_h(p1_nm, 256, hn2_fm, h)
                pool_lin_h(p1_fm, hn2_fm, 512, qW2, BC_QB2, R_QB2, True, 8,
                           p2_fm, p2_nm, h)
                pool_hn_h(p2_nm, 256, hn3_fm, h)
                pool_lin_h(p2_fm, hn3_fm, 512, qW3, BC_QB3, R_QB3, False, 10,
                           p3_fm, None, h)

        ex5b.close()
        ex5.close()

        # ---------- final MLP ----------
        for co in range(2):
            ps = ps_sml(128, G)
            for k in range(12):
                nc.tensor.matmul(
                    ps[:], lhsT=mW1[:, k * 256 + co * 128 : k * 256 + co * 128 + 128],
                    rhs=out_fm[:, k * G : (k + 1) * G],
                    start=(k == 0), stop=(k == 11))
            nc.scalar.activation(y_sb[:, co * G : (co + 1) * G], ps[:], AF.Identity,
                                 bias=bcol[:, BC_MB1 + co : BC_MB1 + co + 1])
        zps = ps_sml(10, G)
        for ci in range(2):
            nc.tensor.matmul(zps[:], lhsT=mW2[:, ci * 10 : (ci + 1) * 10],
                             rhs=y_sb[:, ci * G : (ci + 1) * G],
                             start=(ci == 0), stop=(ci == 1))
        nc.scalar.activation(z_sb[:], zps[:], AF.Identity,
                             bias=bcol[0:10, BC_MB2 : BC_MB2 + 1])
        nc.sync.dma_start(yp_d[:], z_sb[:])

    nc.compile()
    return nc


# ---------------------------------------------------------------------------
# host side
# ---------------------------------------------------------------------------

def _pack_bcol(b, pb_lc):
    bc = np.zeros((128, BC_N), np.float32)
    for off, k in ((BC_B1, "b1"), (BC_B2, "b2"), (BC_B3, "b3"), (BC_AB1, "ab1"),
                   (BC_AB2, "ab2"), (BC_AB3, "ab3"), (BC_QB1, "qb1"),
                   (BC_QB2, "qb2"), (BC_QB3, "qb3"), (BC_MB1, "mb1")):
        v = np.asarray(b[k], np.float32)
        bc[:, off : off + v.size // 128] = v.reshape(-1, 128).T
    mb2 = np.asarray(b["mb2"], np.float32)
    bc[: mb2.size, BC_MB2] = mb2
    bc[0:64, BC_PB : BC_PB + G] = pb_lc.reshape(G, K).T
    return bc


def tf32_round(v):
    u = np.ascontiguousarray(np.asarray(v, np.float32)).view(np.uint32).copy()
    u &= np.uint32(0xFFFFE000)
    return u.view(np.float32)


def _pack_rows(b):
    r = np.zeros((65, ROWS_W), np.float32)
    for p in (0, 32, 64):
        r[p, 0:512] = 1.0
    for (p, off), k in ((R_QB1, "qb1"), (R_QB2, "qb2"), (R_QB3, "qb3")):
        r[p, off : off + 256] = b[k]
    return tf32_round(r)


def _edge_tables(edge_src, edge_dst, core):
    """Dedup'd (dst, count) tables per (src-tile, partition) for one core."""
    lo, hi = core * NLOC, (core + 1) * NLOC
    m = (edge_dst >= lo) & (edge_dst < hi)
    src = edge_src[m].astype(np.int64)
    dst = edge_dst[m].astype(np.int64)
    gg = dst // NPG
    if not np.array_equal(src // NPG, gg):
        raise ValueError("cross-graph edges break graph-parallel sharding")
    gl = gg - core * G
    sl = src - gg * NPG
    dl = dst - gg * NPG
    t = gl * 4 + sl // 128
    p = sl % 128
    key = (t * 128 + p) * NPG + dl
    uk, cnt = np.unique(key, return_counts=True)
    rows = uk // NPG
    cols = uk % NPG
    nrow = np.bincount(rows, minlength=T * 128)
    if nrow.max() > NI:
        raise ValueError(f"out-degree {nrow.max()} exceeds NI={NI}")
    starts = np.zeros(T * 128, np.int64)
    np.cumsum(nrow[:-1], out=starts[1:])
    pos = np.arange(uk.size) - starts[rows]
    at_idx = np.full((128, T * NI), -1, np.int16)
    at_val = np.zeros((128, T * NI), np.float32)
    pr = (rows % 128).astype(np.int64)
    tr = (rows // 128).astype(np.int64)
    at_idx[pr, tr * NI + pos] = cols.astype(np.int16)
    at_val[pr, tr * NI + pos] = cnt.astype(np.float32)
    # clamped in-degree per local node (exact integer counts)
    degl = np.bincount(dst - lo, minlength=NLOC).astype(np.float32)
    degl = np.maximum(degl, 1.0)
    return at_idx, at_val.astype(ml_dtypes.bfloat16), degl


_GSLOT = [(0, 0), (32, 0), (64, 0), (32, 512)]
_CACHE = {}
TRACE = False


def prepare_in_maps(inputs):
    f32 = lambda x: np.ascontiguousarray(np.asarray(x, np.float32))
    bf16 = lambda x: np.ascontiguousarray(
        np.asarray(x, np.float32).astype(ml_dtypes.bfloat16))
    feat = f32(inputs["feat"])
    edge_src = np.asarray(inputs["edge_src"])
    edge_dst = np.asarray(inputs["edge_dst"])
    W = {k: tf32_round(inputs[k]) for k in
         ("W1", "W2", "W3", "aW1", "mW1", "mW2")}
    Wb = {k: bf16(inputs[k]) for k in
          ("aW2", "qW1", "qW2", "qW3")}
    pW64 = np.asarray(inputs["pW"], np.float32) * 64.0
    Wb["pW12"] = np.ascontiguousarray(pW64[0:512] * 16.0).astype(
        ml_dtypes.bfloat16)
    Wb["pW3"] = np.ascontiguousarray(pW64[512:2560]).astype(
        ml_dtypes.float8_e4m3fn)
    Wb["aW3"] = np.ascontiguousarray(
        (np.asarray(inputs["aW3"], np.float32) * 16.0).astype(
            ml_dtypes.float8_e4m3fn))
    b = {k: f32(inputs[k]) for k in
         ("b1", "b2", "b3", "ab1", "ab2", "ab3", "pb", "qb1", "qb2", "qb3",
          "mb1", "mb2")}
    b["ab3"] = b["ab3"] * 16.0  # matches the x16 fp8 aW3 (a3p holds 16*a3)
    ident = np.eye(128, dtype=np.float32)
    rows2 = _pack_rows(b)

    in_maps = []
    for c in range(NCORES):
        fs = feat[c * NLOC : (c + 1) * NLOC]
        feat_nm = np.ascontiguousarray(
            fs.reshape(T, 128, IN).transpose(1, 0, 2).reshape(128, T * IN))
        featT = np.ascontiguousarray(fs.T)
        at_idx, at_val, degl = _edge_tables(edge_src, edge_dst, c)
        pW12_lc = np.ascontiguousarray(Wb["pW12"][:, c * G * K : (c + 1) * G * K])
        pW3_lc = np.ascontiguousarray(Wb["pW3"][:, c * G * K : (c + 1) * G * K])
        pb_lc = np.ascontiguousarray(b["pb"][c * G * K : (c + 1) * G * K])
        degr = np.zeros((65, 1024), np.float32)
        for g in range(G):
            p, off = _GSLOT[g]
            degr[p, off : off + NPG] = degl[g * NPG : (g + 1) * NPG]
        in_maps.append({
            "featT": tf32_round(featT),
            "feat_nm": tf32_round(feat_nm),
            "at_idx": at_idx, "at_val": at_val,
            "bcol": _pack_bcol(b, pb_lc), "rows2": rows2,
            "ident": ident, "identr": ident,
            "degr": degr,
            "W1": W["W1"], "W2": W["W2"], "W3": W["W3"],
            "aW1": W["aW1"], "aW2": Wb["aW2"], "aW3": Wb["aW3"],
            "pW12": pW12_lc, "pW3": pW3_lc,
            "qW1": Wb["qW1"], "qW2": Wb["qW2"], "qW3": Wb["qW3"],
            "mW1": W["mW1"], "mW2": W["mW2"],
        })
    return in_maps


def kernel(**inputs):
    if "nc" not in _CACHE:
        _CACHE["nc"] = build_module()
    nc = _CACHE["nc"]
    in_maps = prepare_in_maps(inputs)
    res = run_bass_kernel_spmd(nc, in_maps, core_ids=list(range(NCORES)),
                               trace=TRACE)
    _CACHE["last_res"] = res
    out = np.zeros((B, 10), np.float32)
    for c in range(NCORES):
        out[c * G : (c + 1) * G, :] = np.asarray(res.results[c]["yp"]).T
    return out


# revision 64
# speedup vs baseline: 1.7471x; 1.0049x over previous
"""DiffPoolEncoder Trainium2 kernel.

Sharding: data parallel by graph. 8 cores x 4 graphs (512 nodes each).
Per core the whole network runs on-device; GraphSage aggregation uses dense
per-graph A^T tiles built on-device via gpsimd local_scatter from
host-prepared (dst, count) index tables (index-only preprocessing of the
edge list; clamped in-degrees are integer edge counts and ship as exact
f32). The h-branch (readout) runs in float32r; the assignment branch,
pooled stage and all spill-free node-major copies run in bfloat16 --
matmul row rate is identical, but bf16 halves SBUF/DMA and avoids the
f32r 4-cycles-per-row penalty on narrow outputs. Softmax, A@S, S^T X and
S^T A S are fused per-graph into the a3/logits loop so the pooled tail
overlaps the big assignment matmuls. SBUF pools are strictly LIFO-nested
by lifetime epoch.
"""

import sys

for _p in ("/opt/trn_rl_repo",):
    if _p not in sys.path:
        sys.path.append(_p)

import numpy as np
import ml_dtypes
from contextlib import ExitStack

import concourse.bass as bass
import concourse.mybir as mybir
import concourse.tile as tile
from concourse import bacc
from concourse.bass_utils import run_bass_kernel_spmd

F32 = mybir.dt.float32
F32R = mybir.dt.float32r
BF16 = mybir.dt.bfloat16
FP8 = mybir.dt.float8e4
I16 = mybir.dt.int16
AF = mybir.ActivationFunctionType
ALU = mybir.AluOpType
AX = mybir.AxisListType

NCORES = 8
B = 32
NPG = 512
G = 4            # graphs per core
T = 16           # node tiles per core (4 per graph)
NLOC = 2048      # nodes per core
K = 64           # clusters per graph
IN = 128
HID = 256
NI = 48          # padded (dst,count) entries per (src-tile, partition)

# bcol column layout (each 128-chunk of a bias vector is one column)
BC_B1, BC_B2, BC_B3 = 0, 2, 4
BC_AB1, BC_AB2 = 6, 8
BC_AB3 = 10          # 16 cols
BC_QB1, BC_QB2, BC_QB3 = 26, 28, 30
BC_MB1, BC_MB2 = 32, 34
BC_PB = 35           # 4 cols: per-graph local pb, partitions 0:64
BC_N = 39

# rows2 [65, 1536] f32r: rows at matmul base partitions {0, 32, 64};
# ones[0:512] replicated at each used partition (matmul needs equal bases).
R_QB1 = (0, 512)
R_QB2, R_QB3 = (64, 512), (64, 768)
ROWS_W = 1024


def build_module():
    nc = bacc.Bacc("TRN2", target_bir_lowering=False)

    # ---------------- DRAM I/O ----------------
    featT_d = nc.dram_tensor("featT", [128, NLOC], F32R, kind="ExternalInput")
    featnm_d = nc.dram_tensor("feat_nm", [128, T * IN], F32R, kind="ExternalInput")
    atidx_d = nc.dram_tensor("at_idx", [128, T * NI], I16, kind="ExternalInput")
    atval_d = nc.dram_tensor("at_val", [128, T * NI], BF16, kind="ExternalInput")
    bcol_d = nc.dram_tensor("bcol", [128, BC_N], F32, kind="ExternalInput")
    rows_d = nc.dram_tensor("rows2", [65, ROWS_W], F32R, kind="ExternalInput")
    ident_d = nc.dram_tensor("ident", [128, 128], F32, kind="ExternalInput")
    identr_d = nc.dram_tensor("identr", [128, 128], F32R, kind="ExternalInput")
    # clamped in-degree (exact integer counts), per-graph gslot rows
    degr_d = nc.dram_tensor("degr", [65, 1024], F32R, kind="ExternalInput")
    w_d = {}
    for name, fi, fo, dt in [
        ("W1", 256, 256, F32R), ("W2", 512, 256, F32R), ("W3", 512, 256, F32R),
        ("aW1", 256, 256, F32R), ("aW2", 512, 256, BF16),
        ("aW3", 512, 2048, mybir.dt.float8e4),
        ("pW12", 512, 256, BF16), ("pW3", 2048, 256, mybir.dt.float8e4),
        ("qW1", 1536, 256, BF16), ("qW2", 512, 256, BF16),
        ("qW3", 512, 256, BF16), ("mW1", 1536, 256, F32R),
        ("mW2", 256, 10, F32R),
    ]:
        w_d[name] = nc.dram_tensor(name, [fi, fo], dt, kind="ExternalInput")
    yp_d = nc.dram_tensor("yp", [10, G], F32, kind="ExternalOutput")

    with tile.TileContext(nc) as tc, ExitStack() as ex, \
            nc.allow_low_precision(reason="f32r/bf16 matmuls; accumulation stays fp32 in PSUM"):
        persist = ex.enter_context(tc.tile_pool(name="persist", bufs=1))
        # PSUM: 8 banks. One tag per pool so slot count == bank count.
        ps_p = ex.enter_context(tc.tile_pool(name="psP", bufs=3, space="PSUM"))
        lg_p = ex.enter_context(tc.tile_pool(name="psL", bufs=1, space="PSUM"))
        pm_p = ex.enter_context(tc.tile_pool(name="psM", bufs=2, space="PSUM"))
        pl_p = ex.enter_context(tc.tile_pool(name="psS", bufs=2, space="PSUM"))

        uid = [0]

        def _nm(pfx):
            uid[0] += 1
            return f"{pfx}{uid[0]}"

        def ps_big(dt=F32):
            return ps_p.tile([128, 512], dt, tag="ps", name=_nm("ps"))

        def ps_med(p, f, dt=F32):
            return pm_p.tile([p, f], dt, tag="pm", name=_nm("pm"))

        def ps_sml(p, f, dt=F32):
            return pl_p.tile([p, f], dt, tag="pl", name=_nm("pl"))

        def wload(pool, name, fi, fo, dt=F32R):
            kk = fi // 128
            sb = pool.tile([128, kk * fo], dt, tag=name, name=name)
            nc.sync.dma_start(
                sb[:].rearrange("p (k f) -> p k f", k=kk, f=fo),
                w_d[name][:, :].rearrange("(k p) f -> p k f", p=128),
            )
            return sb

        # ---------- persistent small tensors (epoch E6) ----------
        ident = persist.tile([128, 128], F32)
        identr = persist.tile([128, 128], F32R)
        identb = persist.tile([128, 128], BF16)
        bcol = persist.tile([128, BC_N], F32)
        ones_pr = persist.tile([65, 128], F32R)  # ones rows at base partitions
        nmax = persist.tile([128, 2], F32)
        sumx = persist.tile([128, 2], F32)
        nc.vector.memset(ones_pr[:].bitcast(F32), 1.0)

        def ones_at(p, n):
            return ones_pr[p : p + 1, 0:n]

        # right-side stack: tensors produced mid-stream and consumed by the
        # pooled tail; outlives the left-side phase pools.
        keep = ex.enter_context(tc.tile_pool(name="keep", bufs=1, side="right"))
        out_fm = keep.tile([128, 12 * G], F32R)  # readout maxes, col=ch*G+g
        y_sb = keep.tile([128, 2 * G], F32R)
        z_sb = keep.tile([10, G], F32)
        # node-major g_emb layers, bf16, for the pooled stage (no DRAM spill)
        h1nb = keep.tile([128, T * HID], BF16)
        h2nb = keep.tile([128, T * HID], BF16)

        # AT (scaled A^T tiles, f32r) lives until AS (epoch E6)
        at_p = ex.enter_context(tc.tile_pool(name="atp", bufs=1))
        AT = at_p.tile([128, T * NPG], F32R)

        # ---------- LIFO phase pools ----------
        ex5 = ExitStack()   # close after logits/softmax
        agg_p = ex5.enter_context(tc.tile_pool(name="aggfm", bufs=1))
        afm1_p = ex5.enter_context(tc.tile_pool(name="afm1", bufs=1))
        afm2_p = ex5.enter_context(tc.tile_pool(name="afm2", bufs=1))
        ex3 = ExitStack()   # close after h3 (h3 runs after xnm closes)
        xfm_p = ex3.enter_context(tc.tile_pool(name="xfm", bufs=2))
        w3_p = ex3.enter_context(tc.tile_pool(name="w3p", bufs=1))
        ex4 = ExitStack()   # close after agg_a2
        xnm_p = ex4.enter_context(tc.tile_pool(name="xnm", bufs=2))
        ex2c = ExitStack()  # close after a2
        a2w_p = ex2c.enter_context(tc.tile_pool(name="a2wp", bufs=1))
        ex2b = ExitStack()  # close after h2
        w2_p = ex2b.enter_context(tc.tile_pool(name="w2p", bufs=1))
        ex2 = ExitStack()   # close after h1/a1
        w1_p = ex2.enter_context(tc.tile_pool(name="w1p", bufs=1))
        agf_p = ex2.enter_context(tc.tile_pool(name="agfp", bufs=1))
        ex1 = ExitStack()   # close after agg_feat
        fnm_p = ex1.enter_context(tc.tile_pool(name="fnmp", bufs=1))
        ex0 = ExitStack()   # close after A^T built+scaled
        ate_p = ex0.enter_context(tc.tile_pool(name="atep", bufs=1))

        # ---------- input DMAs ----------
        atbf_idx = ate_p.tile([128, T * NI], I16, tag="atidx")
        atbf_val = ate_p.tile([128, T * NI], BF16, tag="atval")
        # DMA priority order: the scatter/scale chain first, then layer-1
        # activations, then constants and weights.
        dinvr = ate_p.tile([65, 1024], F32R, tag="dinvr")  # deg -> 1/deg rows
        nc.sync.dma_start(dinvr[:], degr_d[:])
        nc.sync.dma_start(atbf_idx[:], atidx_d[:])
        nc.sync.dma_start(atbf_val[:], atval_d[:])
        featnm = fnm_p.tile([128, T * IN], F32R, tag="featnm")
        nc.sync.dma_start(featnm[:], featnm_d[:])
        ftg = []
        for g in range(G):
            ft = fnm_p.tile([128, NPG], F32R, tag="ftc", name=_nm("ftc"),
                            bufs=2)
            nc.sync.dma_start(ft[:], featT_d[:, g * NPG : (g + 1) * NPG])
            ftg.append(ft)
        nc.sync.dma_start(ident[:], ident_d[:])
        nc.sync.dma_start(identr[:], identr_d[:])
        nc.sync.dma_start(bcol[:], bcol_d[:])
        W1 = wload(w1_p, "W1", 256, 256)
        aW1 = wload(w1_p, "aW1", 256, 256)
        W2 = wload(w2_p, "W2", 512, 256)
        W3 = wload(w3_p, "W3", 512, 256)
        aW2 = wload(a2w_p, "aW2", 512, 256, BF16)

        gslot = [(0, 0), (32, 0), (64, 0), (32, 512)]
        for g in range(G):
            p, off = gslot[g]
            nc.vector.reciprocal(dinvr[p : p + 1, off : off + NPG],
                                 dinvr[p : p + 1, off : off + NPG])

        # ---------- emit helpers ----------
        def emit_agg(x_nm, D, out_t, glist=range(G)):
            """out_t[d, n] (feature-major) = sum_s x_nm[s, d] * AT[s, n]."""
            for g in glist:
                for ch in range(D // 128):
                    ps = ps_big()
                    for st in range(4):
                        t = g * 4 + st
                        nc.tensor.matmul(
                            ps[:],
                            lhsT=x_nm[:, t * D + ch * 128 : t * D + ch * 128 + 128],
                            rhs=AT[:, t * NPG : (t + 1) * NPG],
                            start=(st == 0), stop=(st == 3))
                    # AT holds raw edge counts; the mean-normalization by
                    # 1/deg[dst] rides along with the DVE evac for free
                    nc.vector.tensor_tensor(
                        out=out_t[:, ch * NLOC + g * NPG : ch * NLOC + (g + 1) * NPG],
                        in0=ps[:], in1=dbc[:, g * NPG : (g + 1) * NPG],
                        op=ALU.mult)

        def emit_lin_fm(x_fm, a_fm, Din, Dout, Wsb, bccol, relu, out_t,
                        nblist=range(4)):
            nk = Din // 128
            for co in range(Dout // 128):
                for nb in nblist:
                    ps = ps_big()
                    ki = 0
                    for src in (x_fm, a_fm):
                        for ci in range(nk):
                            nc.tensor.matmul(
                                ps[:],
                                lhsT=Wsb[:, ki * Dout + co * 128 : ki * Dout + co * 128 + 128],
                                rhs=src[:, ci * NLOC + nb * 512 : ci * NLOC + (nb + 1) * 512],
                                start=(ki == 0), stop=(ki == 2 * nk - 1))
                            ki += 1
                    nc.scalar.activation(
                        out_t[:, co * NLOC + nb * 512 : co * NLOC + (nb + 1) * 512],
                        ps[:], AF.Relu if relu else AF.Identity,
                        bias=bcol[:, bccol + co : bccol + co + 1])

        def emit_nm_T(x_fm, out_nm, dt=F32R, bcopy=None, t2list=None):
            # node-major via PE transposes of the (already relu'd) fm tensor;
            # 4 transposed blocks share one PSUM bank -> single 512-wide evac.
            # bcopy: persistent bf16 copy evacuated on the idle Pool engine.
            idm = identb if dt == BF16 else identr
            for t2 in (range(0, T, 2) if t2list is None else t2list):
                tp = ps_big(dt)
                for i, (t, ch) in enumerate(
                        ((t2, 0), (t2, 1), (t2 + 1, 0), (t2 + 1, 1))):
                    nc.tensor.matmul(
                        tp[:, i * 128 : (i + 1) * 128],
                        lhsT=x_fm[:, ch * NLOC + t * 128 : (ch * NLOC + t * 128) + 128],
                        rhs=idm[:], is_transpose=True,
                        start=True, stop=True, skip_group_check=True)
                sl = slice(t2 * HID, (t2 + 2) * HID)
                if out_nm is not None:
                    if dt == BF16:
                        nc.scalar.copy(out_nm[:, sl], tp[:])
                    else:
                        nc.vector.tensor_copy(out_nm[:, sl], tp[:])
                    if bcopy is not None:
                        # bf16 shadow for the pooled tail; SBUF->SBUF on the
                        # idle Pool engine (GPSIMD cannot read PSUM)
                        nc.gpsimd.tensor_copy(bcopy[:, sl], out_nm[:, sl])
                elif bcopy is not None:
                    nc.scalar.copy(bcopy[:, sl], tp[:])

        def emit_out1(x_fm, ch0, glist=range(G)):
            for ci in range(2):
                for g in glist:
                    nc.vector.tensor_reduce(
                        out_fm[:, (ch0 + ci) * G + g : (ch0 + ci) * G + g + 1],
                        x_fm[:, ci * NLOC + g * NPG : ci * NLOC + (g + 1) * NPG],
                        axis=AX.X, op=ALU.max)

        # ---------- phase 0 fused with h1/a1, pipelined per graph ----------
        aggfeat = agf_p.tile([128, NLOC], F32R, tag="aggf", name="aggfeat")
        h1f = xfm_p.tile([128, 2 * NLOC], F32R, tag="xfm", name="h1f")
        h1n = xnm_p.tile([128, T * HID], F32R, tag="xnm", name="h1n")
        a1f = afm1_p.tile([128, 2 * NLOC], BF16, tag="a1f", name="a1f")
        a1n = xnm_p.tile([128, T * HID], F32R, tag="xnm", name="a1n")

        def emit_lin1(ftc, g, Wsb, bccol, out_t):
            # layer-1 linear for one graph: cat(feat, agg_feat) @ W
            for co in range(2):
                ps = ps_big()
                nc.tensor.matmul(
                    ps[:], lhsT=Wsb[:, co * 128 : co * 128 + 128],
                    rhs=ftc[:], start=True, stop=False)
                nc.tensor.matmul(
                    ps[:], lhsT=Wsb[:, 256 + co * 128 : 256 + co * 128 + 128],
                    rhs=aggfeat[:, g * NPG : (g + 1) * NPG],
                    start=False, stop=True)
                nc.scalar.activation(
                    out_t[:, co * NLOC + g * NPG : co * NLOC + (g + 1) * NPG],
                    ps[:], AF.Relu,
                    bias=bcol[:, bccol + co : bccol + co + 1])

        # A^T build first for ALL graphs: keeps the in-order Pool queue
        # (scatters) free of later-phase ops, so graph g+1's scatter never
        # waits behind graph g's h1 consumers. AT keeps RAW counts; the
        # 1/deg broadcast tiles (dbc) are applied at every agg evac instead.
        dbc = agg_p.tile([128, G * NPG], F32R, tag="dbc", name="dbc")
        for g in range(G):
            p, off = gslot[g]
            bps = ps_big()
            nc.tensor.matmul(bps[:], lhsT=ones_at(p, 128),
                             rhs=dinvr[p : p + 1, off : off + NPG],
                             start=True, stop=True)
            nc.scalar.copy(dbc[:, g * NPG : (g + 1) * NPG], bps[:])
        for t in range(T):
            scr = ate_p.tile([128, NPG], BF16, tag="scscr", name=_nm("sc"),
                             bufs=2)
            nc.gpsimd.local_scatter(
                out_ap=scr[:],
                data_ap=atbf_val[:, t * NI : (t + 1) * NI],
                idxs_ap=atbf_idx[:, t * NI : (t + 1) * NI],
                channels=128, num_elems=NPG, num_idxs=NI,
            )
            # raw A^T tile; alternate evac engine to balance Act/DVE
            if t % 2 == 0:
                nc.scalar.copy(AT[:, t * NPG : (t + 1) * NPG], scr[:])
            else:
                nc.vector.tensor_copy(AT[:, t * NPG : (t + 1) * NPG], scr[:])
        nc.vector.tensor_copy(identb[:], ident[:])
        for g in range(G):
            emit_agg(featnm, IN, aggfeat, glist=[g])
            emit_lin1(ftg[g], g, W1, BC_B1, h1f)
            emit_lin1(ftg[g], g, aW1, BC_AB1, a1f)
            emit_nm_T(h1f, h1n, bcopy=h1nb, t2list=(4 * g, 4 * g + 2))
            emit_nm_T(a1f, a1n, dt=BF16, t2list=(4 * g, 4 * g + 2))
        emit_out1(h1f, 0)
        ex0.close()
        ex1.close()
        ex2.close()

        aggh1 = agg_p.tile([128, 2 * NLOC], F32R, tag="aggh", name="aggh1")
        emit_agg(h1n, HID, aggh1)

        agga1 = agg_p.tile([128, 2 * NLOC], BF16, tag="agga", name="agga1")
        emit_agg(a1n, HID, agga1)

        h2f = xfm_p.tile([128, 2 * NLOC], F32R, tag="xfm", name="h2f")
        h2n = xnm_p.tile([128, T * HID], F32R, tag="xnm", name="h2n")
        emit_lin_fm(h1f, aggh1, 256, 256, W2, BC_B2, True, h2f)
        emit_nm_T(h2f, h2n, bcopy=h2nb)
        emit_out1(h2f, 2)
        ex2b.close()

        a2f = afm2_p.tile([128, 2 * NLOC], BF16, tag="a2f", name="a2f")
        a2n = xnm_p.tile([128, T * HID], F32R, tag="xnm", name="a2n")
        emit_lin_fm(a1f, agga1, 256, 256, aW2, BC_AB2, True, a2f)
        emit_nm_T(a2f, a2n, dt=BF16)
        ex2c.close()

        aggh2 = agg_p.tile([128, 2 * NLOC], F32R, tag="aggh", name="aggh2")
        emit_agg(h2n, HID, aggh2)

        agga2 = agg_p.tile([128, 2 * NLOC], BF16, tag="agga", name="agga2")
        emit_agg(a2n, HID, agga2)
        ex4.close()

        # pooled-stage tiles + weights: prefetch during the a3/logits phase.
        # Right-side SBUF stack: outlives the left-side phase pools without
        # violating their LIFO discipline.
        late = ex.enter_context(tc.tile_pool(name="late", bufs=1, side="right"))
        rows2 = late.tile([65, ROWS_W], F32R, tag="rows2", name="rows2")
        nc.sync.dma_start(rows2[:], rows_d[:])

        def rrow(ro, n):
            p, off = ro
            return rows2[p : p + 1, off : off + n]

        qW1 = wload(late, "qW1", 1536, 256, BF16)
        # fp8 shadows of the a3 operands (DVE converting copies, emitted
        # here so they overlap h3 instead of stalling the a3 loop entry)
        a2f8 = late.tile([128, 2 * NLOC], FP8, tag="a2f8", name="a2f8")
        agga28 = late.tile([128, 2 * NLOC], FP8, tag="agga28", name="agga28")
        for ci in range(2):
            nc.vector.tensor_copy(a2f8[:, ci * NLOC : (ci + 1) * NLOC],
                                  a2f[:, ci * NLOC : (ci + 1) * NLOC])
            nc.vector.tensor_copy(agga28[:, ci * NLOC : (ci + 1) * NLOC],
                                  agga2[:, ci * NLOC : (ci + 1) * NLOC])
        mW1 = wload(late, "mW1", 1536, 256)
        mW2 = wload(late, "mW2", 256, 10)
        qW2 = wload(late, "qW2", 512, 256, BF16)
        qW3 = wload(late, "qW3", 512, 256, BF16)
        S_b = late.tile([128, T * K], BF16, tag="S_b", name="S_b")
        h3nb = late.tile([128, T * HID], BF16, tag="h3nb", name="h3nb")
        AS_nm = late.tile([128, T * K], BF16, tag="AS", name="AS_nm")
        hp_nm = late.tile([128, 2 * 768], BF16, tag="hpn", name="hp_nm")
        hp_fm = late.tile([128, 6 * 256], BF16, tag="hpf", name="hp_fm")
        adjg = late.tile([128, 2 * K], F32, tag="adjg", name="adjg")
        rsum = late.tile([128, 2], F32, tag="rsum", name="rsum")
        adjT = late.tile([128, 2 * 128], BF16, tag="adjT", name="adjT")
        hn1_fm = late.tile([128, 6 * 256], BF16, tag="hn1", name="hn1_fm")
        p1_nm = late.tile([128, 2 * 256], BF16, tag="p1n", name="p1_nm")
        p1_fm = late.tile([128, 2 * 256], BF16, tag="p1f", name="p1_fm")
        hn2_fm = late.tile([128, 2 * 256], BF16, tag="hn2", name="hn2_fm")
        p2_nm = late.tile([128, 2 * 256], BF16, tag="p2n", name="p2_nm")
        p2_fm = late.tile([128, 2 * 256], BF16, tag="p2f", name="p2_fm")
        hn3_fm = late.tile([128, 2 * 256], BF16, tag="hn3", name="hn3_fm")
        p3_fm = late.tile([128, 2 * 256], BF16, tag="p3f", name="p3_fm")
        nc.vector.memset(adjT[:].bitcast(F32), 0.0)
        Xr = [h1nb, h2nb, h3nb]

        # h3: fm + readout; node-major -> persistent bf16 only
        h3f = xfm_p.tile([128, 2 * NLOC], F32R, tag="xfm", name="h3f")
        emit_lin_fm(h2f, aggh2, 256, 256, W3, BC_B3, False, h3f)
        emit_nm_T(h3f, None, bcopy=h3nb)
        emit_out1(h3f, 4)
        ex3.close()

        # a3/logits weights on the right stack, opened once xfm/w3 are gone
        ex5b = ExitStack()
        wl_p = ex5b.enter_context(tc.tile_pool(name="wlate", bufs=1,
                                               side="right"))
        aW3 = wl_p.tile([128, 4 * 2048], FP8, tag="aW3", name="aW3")
        aW3v = aW3[:].rearrange("p (k f) -> p k f", k=4, f=2048)
        for q in range(4):
            nc.sync.dma_start(
                aW3v[:, :, q * 512 : (q + 1) * 512],
                w_d["aW3"][:, q * 512 : (q + 1) * 512].rearrange(
                    "(k p) f -> p k f", p=128))
        pW12 = wload(wl_p, "pW12", 512, 256, BF16)
        pW3 = wload(wl_p, "pW3", 2048, 256, mybir.dt.float8e4)

        # ---------- a3 + logits + softmax + AS + pooled prep, per graph ----
        def emit_hpool(h):
            # both graphs of the pair into one PSUM tile via partition-offset
            # outputs (base partition 64 for the odd graph) -> single evac
            for L in range(3):
                ps = ps_med(128, 256)
                for gs in range(2):
                    g = h * 2 + gs
                    for j in range(4):
                        t = g * 4 + j
                        nc.tensor.matmul(
                            ps[gs * 64 : gs * 64 + 64, :],
                            lhsT=S_b[:, t * K : (t + 1) * K],
                            rhs=Xr[L][:, t * HID : (t + 1) * HID],
                            start=(j == 0), stop=(j == 3),
                            skip_group_check=True)
                nc.vector.tensor_copy(
                    hp_nm[:, h * 768 + L * 256 : h * 768 + (L + 1) * 256],
                    ps[:])
            for ch in range(6):  # hp_fm via transposes of the pair tile
                tp = ps_med(128, 128, BF16)
                nc.tensor.matmul(
                    tp[:], lhsT=hp_nm[:, h * 768 + ch * 128 : h * 768 + (ch + 1) * 128],
                    rhs=identb[:], is_transpose=True,
                    start=True, stop=True, skip_group_check=True)
                nc.vector.tensor_copy(
                    hp_fm[:, ch * 256 + h * 128 : ch * 256 + (h + 1) * 128], tp[:])

        def emit_adj(h):
            ps = ps_sml(128, K)
            for gs in range(2):
                g = h * 2 + gs
                for j in range(4):
                    t = g * 4 + j
                    nc.tensor.matmul(ps[gs * 64 : gs * 64 + 64, :],
                                     lhsT=S_b[:, t * K : (t + 1) * K],
                                     rhs=AS_nm[:, t * K : (t + 1) * K],
                                     start=(j == 0), stop=(j == 3),
                                     skip_group_check=True)
            nc.vector.tensor_copy(adjg[:, h * K : (h + 1) * K], ps[:])
            nc.vector.tensor_reduce(rsum[:, h : h + 1], adjg[:, h * K : (h + 1) * K],
                                    axis=AX.X, op=ALU.add)
            nc.vector.tensor_scalar(rsum[:, h : h + 1], rsum[:, h : h + 1],
                                    1e-9, None, op0=ALU.add)
            nc.vector.reciprocal(rsum[:, h : h + 1], rsum[:, h : h + 1])
            nc.vector.tensor_scalar(adjg[:, h * K : (h + 1) * K],
                                    adjg[:, h * K : (h + 1) * K],
                                    rsum[:, h : h + 1], None, op0=ALU.mult)
            # transpose each graph's [64,64] block onto the block diagonal
            # (transpose outputs must land at PSUM partition 0; odd block is
            # partition-shifted into place with a small SBUF->SBUF DMA)
            for gs in range(2):
                tp = ps_sml(128, K)
                nc.tensor.transpose(
                    tp[0:64, :],
                    adjg[gs * 64 : gs * 64 + 64, h * K : (h + 1) * K],
                    ident[gs * 64 : gs * 64 + 64, gs * 64 : gs * 64 + 64]
                    if gs else ident[0:64, 0:64])
                if gs == 0:
                    nc.vector.tensor_copy(adjT[0:64, h * 128 : h * 128 + 64],
                                          tp[0:64, :])
                else:
                    sb = late.tile([64, K], BF16, tag="adjsh", name=_nm("adjsh"),
                                   bufs=2)
                    nc.vector.tensor_copy(sb[:], tp[0:64, :])
                    nc.sync.dma_start(
                        adjT[64:128, h * 128 + 64 : h * 128 + 128], sb[:])

        def pool_hn_h(x_nm, xw, out_t, h):
            # out[d, u] = sum_v x_nm[v, d] * adjT_bd[v, u], one graph pair
            for ch in range(xw // 128):
                tp = ps_sml(128, 128)
                nc.tensor.matmul(
                    tp[:],
                    lhsT=x_nm[:, h * xw + ch * 128 : h * xw + (ch + 1) * 128],
                    rhs=adjT[:, h * 128 : (h + 1) * 128],
                    start=True, stop=True)
                nc.vector.tensor_copy(
                    out_t[:, ch * 256 + h * 128 : ch * 256 + (h + 1) * 128],
                    tp[:])

        def pool_lin_h(xf, hf, Din, Wsb, bccol, rbias, relu, ch0, outf, outn,
                       h):
            nch = Din // 256
            for co in range(2):
                ps = ps_med(128, 128)
                ki = 0
                for src in (xf, hf):
                    for ch in range(nch):
                        nc.tensor.matmul(
                            ps[:],
                            lhsT=Wsb[:, ki * 256 + co * 128 : ki * 256 + co * 128 + 128],
                            rhs=src[:, ch * 256 + h * 128 : ch * 256 + (h + 1) * 128],
                            start=(ki == 0), stop=(ki == 2 * nch - 1))
                        ki += 1
                nc.scalar.activation(
                    outf[:, co * 256 + h * 128 : co * 256 + (h + 1) * 128],
                    ps[:], AF.Relu if relu else AF.Identity,
                    bias=bcol[:, bccol + co : bccol + co + 1])
                for gs in range(2):
                    g2 = 2 * h + gs
                    nc.vector.tensor_reduce(
                        out_fm[:, (ch0 + co) * G + g2 : (ch0 + co) * G + g2 + 1],
                        outf[:, co * 256 + h * 128 + gs * K : co * 256 + h * 128 + (gs + 1) * K],
                        axis=AX.X, op=ALU.max)
            if outn is not None:
                ps = ps_med(128, 256)
                ki = 0
                for src in (xf, hf):
                    for ch in range(nch):
                        nc.tensor.matmul(
                            ps[:],
                            lhsT=src[:, ch * 256 + h * 128 : ch * 256 + (h + 1) * 128],
                            rhs=Wsb[:, ki * 256 : (ki + 1) * 256],
                            start=(ki == 0), stop=False)
                        ki += 1
                nc.tensor.matmul(ps[:], lhsT=ones_at(rbias[0], 128).bitcast(F32),
                                 rhs=rrow(rbias, 256).bitcast(F32),
                                 start=False, stop=True)
                nc.vector.tensor_scalar(outn[:, h * 256 : (h + 1) * 256], ps[:],
                                        0.0, None, op0=ALU.max)

        for g in range(G):
            lps = lg_p.tile([64, 512], F32, tag="lg", name=_nm("lg"))
            a3p = None
            for co in range(16):  # a3 = relu(cat(a2, agg_a2) @ aW3 + ab3)
                # fp8e4m3 DoubleRow: two 128-deep k-tiles per matmul at
                # 0.5 cycles/row; weights pre-scaled x64 on host, undone by
                # the evac's activation scale. The relu'd a3 is kept in fp8
                # pair-tiles so the logits a3-block also runs DoubleRow
                # (pW3 is x64 fp8; the lgf evac divides the sum back down).
                ps3 = ps_big()
                aw = aW3[:].rearrange("p (k f) -> p k f", k=4, f=2048)
                for q, src8 in enumerate((a2f8, agga28)):
                    nc.tensor.matmul(
                        ps3[:],
                        lhsT=aw[:, 2 * q : 2 * q + 2,
                                co * 128 : co * 128 + 128],
                        rhs=src8[:].rearrange("p (k n) -> p k n", k=2,
                                              n=NLOC)[:, :, g * NPG : (g + 1) * NPG],
                        start=(q == 0), stop=(q == 1),
                        perf_mode=mybir.MatmulPerfMode.DoubleRow)
                if co % 2 == 0:
                    a3p = wl_p.tile([128, 2 * 512], mybir.dt.float8e4,
                                    tag="a3p", name=_nm("a3p"), bufs=2)
                dst8 = a3p[:, (co % 2) * 512 : (co % 2 + 1) * 512]
                if co % 2 == 0:
                    nc.scalar.activation(
                        dst8, ps3[:], AF.Relu,
                        bias=bcol[:, BC_AB3 + co : BC_AB3 + co + 1])
                else:
                    nc.vector.tensor_scalar(
                        dst8, ps3[:],
                        bcol[:, BC_AB3 + co : BC_AB3 + co + 1], 0.0,
                        op0=ALU.add, op1=ALU.max)
                if co % 2 == 1:
                    qq = co // 2
                    nc.tensor.matmul(
                        lps[:],
                        lhsT=pW3[:].rearrange("p (k f) -> p k f", k=16,
                                              f=256)[:, 2 * qq : 2 * qq + 2,
                                                     g * K : g * K + K],
                        rhs=a3p[:].rearrange("p (k n) -> p k n", k=2, n=512),
                        start=(qq == 0), stop=False,
                        perf_mode=mybir.MatmulPerfMode.DoubleRow)
            for ci in range(2):  # a1 block of pW
                nc.tensor.matmul(
                    lps[:], lhsT=pW12[:, ci * 256 + g * K : ci * 256 + g * K + K],
                    rhs=a1f[:, ci * NLOC + g * NPG : ci * NLOC + (g + 1) * NPG],
                    start=False, stop=False)
            for ci in range(2):  # a2 block
                nc.tensor.matmul(
                    lps[:], lhsT=pW12[:, (2 + ci) * 256 + g * K : (2 + ci) * 256 + g * K + K],
                    rhs=a2f[:, ci * NLOC + g * NPG : ci * NLOC + (g + 1) * NPG],
                    start=False, stop=(ci == 1))
            lgf = wl_p.tile([64, 512], F32, tag="lgf", name=_nm("lgf"), bufs=2)
            nc.scalar.activation(lgf[:], lps[:], AF.Identity, scale=1.0 / 1024,
                                 bias=bcol[0:64, BC_PB + g : BC_PB + g + 1])
            lgt = wl_p.tile([128, 4 * K], F32, tag="lgs", name=_nm("lgs"),
                            bufs=2)
            for j in range(4):  # transpose to node-major
                tps = ps_med(128, 64)
                nc.tensor.transpose(tps[:], lgf[0:64, j * 128 : (j + 1) * 128],
                                    ident[0:64, 0:64])
                nc.vector.tensor_copy(lgt[:, j * K : (j + 1) * K], tps[:])

            # masked softmax == per-graph softmax over K columns. S kept in
            # two forms: transient f32r (for the f32r AS matmuls against AT)
            # and persistent bf16 (for the bf16 pooled stage) — the compiler
            # rejects 32-bit x 16-bit matmul operand mixes.
            S32 = wl_p.tile([128, 4 * K], F32R, tag="S32", name=_nm("S32"),
                            bufs=2)
            for j in range(4):
                t = g * 4 + j
                bb = t % 2
                nc.vector.tensor_reduce(nmax[:, bb : bb + 1],
                                        lgt[:, j * K : (j + 1) * K],
                                        axis=AX.X, op=ALU.max, negate=True)
                nc.scalar.activation(S32[:, j * K : (j + 1) * K],
                                     lgt[:, j * K : (j + 1) * K], AF.Exp,
                                     bias=nmax[:, bb : bb + 1],
                                     accum_out=sumx[:, bb : bb + 1])
                nc.vector.reciprocal(sumx[:, bb : bb + 1], sumx[:, bb : bb + 1])
                nc.vector.tensor_scalar(S32[:, j * K : (j + 1) * K],
                                        S32[:, j * K : (j + 1) * K],
                                        sumx[:, bb : bb + 1], None, op0=ALU.mult)
                nc.gpsimd.tensor_copy(S_b[:, t * K : (t + 1) * K],
                                      S32[:, j * K : (j + 1) * K])

            # AS = A @ S: scaled-AT product un-scaled by clamped deg (exact)
            for j in range(4):
                t = g * 4 + j
                ps = ps_sml(128, K)
                for st in range(4):
                    nc.tensor.matmul(
                        ps[:],
                        lhsT=AT[:, (g * 4 + st) * NPG + j * 128 : (g * 4 + st) * NPG + (j + 1) * 128],
                        rhs=S32[:, st * K : (st + 1) * K],
                        start=(st == 0), stop=(st == 3))
                nc.vector.tensor_copy(AS_nm[:, t * K : (t + 1) * K], ps[:])
            if g % 2 == 1:  # pooled stack for the completed pair
                h = g // 2
                emit_hpool(h)
                emit_adj(h)
                pool_hn_h(hp_nm, 768, hn1_fm, h)
                pool_lin_h(hp_fm, hn1_fm, 1536, qW1, BC_QB1, R_QB1, True, 6,
                           p1_fm, p1_nm, h)
                pool_hn_h(p1_nm, 256, hn2_fm, h)
                pool_lin_h(p1_fm, hn2_fm, 512, qW2, BC_QB2, R_QB2, True, 8,
                           p2_fm, p2_nm, h)
                pool_hn_h(p2_nm, 256, hn3_fm, h)
                pool_lin_h(p2_fm, hn3_fm, 512, qW3, BC_QB3, R_QB3, False, 10,
                           p3_fm, None, h)

        ex5b.close()
        ex5.close()

        # ---------- final MLP ----------
        for co in range(2):
            ps = ps_sml(128, G)
            for k in range(12):
                nc.tensor.matmul(
                    ps[:], lhsT=mW1[:, k * 256 + co * 128 : k * 256 + co * 128 + 128],
                    rhs=out_fm[:, k * G : (k + 1) * G],
                    start=(k == 0), stop=(k == 11))
            nc.scalar.activation(y_sb[:, co * G : (co + 1) * G], ps[:], AF.Identity,
                                 bias=bcol[:, BC_MB1 + co : BC_MB1 + co + 1])
        zps = ps_sml(10, G)
        for ci in range(2):
            nc.tensor.matmul(zps[:], lhsT=mW2[:, ci * 10 : (ci + 1) * 10],
                             rhs=y_sb[:, ci * G : (ci + 1) * G],
                             start=(ci == 0), stop=(ci == 1))
        nc.scalar.activation(z_sb[:], zps[:], AF.Identity,
                             bias=bcol[0:10, BC_MB2 : BC_MB2 + 1])
        nc.sync.dma_start(yp_d[:], z_sb[:])

    nc.compile()
    return nc


# ---------------------------------------------------------------------------
# host side
# ---------------------------------------------------------------------------

def _pack_bcol(b, pb_lc):
    bc = np.zeros((128, BC_N), np.float32)
    for off, k in ((BC_B1, "b1"), (BC_B2, "b2"), (BC_B3, "b3"), (BC_AB1, "ab1"),
                   (BC_AB2, "ab2"), (BC_AB3, "ab3"), (BC_QB1, "qb1"),
                   (BC_QB2, "qb2"), (BC_QB3, "qb3"), (BC_MB1, "mb1")):
        v = np.asarray(b[k], np.float32)
        bc[:, off : off + v.size // 128] = v.reshape(-1, 128).T
    mb2 = np.asarray(b["mb2"], np.float32)
    bc[: mb2.size, BC_MB2] = mb2
    bc[0:64, BC_PB : BC_PB + G] = pb_lc.reshape(G, K).T
    return bc


def tf32_round(v):
    u = np.ascontiguousarray(np.asarray(v, np.float32)).view(np.uint32).copy()
    u &= np.uint32(0xFFFFE000)
    return u.view(np.float32)


def _pack_rows(b):
    r = np.zeros((65, ROWS_W), np.float32)
    for p in (0, 32, 64):
        r[p, 0:512] = 1.0
    for (p, off), k in ((R_QB1, "qb1"), (R_QB2, "qb2"), (R_QB3, "qb3")):
        r[p, off : off + 256] = b[k]
    return tf32_round(r)


def _edge_tables(edge_src, edge_dst, core):
    """Dedup'd (dst, count) tables per (src-tile, partition) for one core."""
    lo, hi = core * NLOC, (core + 1) * NLOC
    m = (edge_dst >= lo) & (edge_dst < hi)
    src = edge_src[m].astype(np.int64)
    dst = edge_dst[m].astype(np.int64)
    gg = dst // NPG
    if not np.array_equal(src // NPG, gg):
        raise ValueError("cross-graph edges break graph-parallel sharding")
    gl = gg - core * G
    sl = src - gg * NPG
    dl = dst - gg * NPG
    t = gl * 4 + sl // 128
    p = sl % 128
    key = (t * 128 + p) * NPG + dl
    uk, cnt = np.unique(key, return_counts=True)
    rows = uk // NPG
    cols = uk % NPG
    nrow = np.bincount(rows, minlength=T * 128)
    if nrow.max() > NI:
        raise ValueError(f"out-degree {nrow.max()} exceeds NI={NI}")
    starts = np.zeros(T * 128, np.int64)
    np.cumsum(nrow[:-1], out=starts[1:])
    pos = np.arange(uk.size) - starts[rows]
    at_idx = np.full((128, T * NI), -1, np.int16)
    at_val = np.zeros((128, T * NI), np.float32)
    pr = (rows % 128).astype(np.int64)
    tr = (rows // 128).astype(np.int64)
    at_idx[pr, tr * NI + pos] = cols.astype(np.int16)
    at_val[pr, tr * NI + pos] = cnt.astype(np.float32)
    # clamped in-degree per local node (exact integer counts)
    degl = np.bincount(dst - lo, minlength=NLOC).astype(np.float32)
    degl = np.maximum(degl, 1.0)
    return at_idx, at_val.astype(ml_dtypes.bfloat16), degl


_GSLOT = [(0, 0), (32, 0), (64, 0), (32, 512)]
_CACHE = {}
TRACE = False


def prepare_in_maps(inputs):
    f32 = lambda x: np.ascontiguousarray(np.asarray(x, np.float32))
    bf16 = lambda x: np.ascontiguousarray(
        np.asarray(x, np.float32).astype(ml_dtypes.bfloat16))
    feat = f32(inputs["feat"])
    edge_src = np.asarray(inputs["edge_src"])
    edge_dst = np.asarray(inputs["edge_dst"])
    W = {k: tf32_round(inputs[k]) for k in
         ("W1", "W2", "W3", "aW1", "mW1", "mW2")}
    Wb = {k: bf16(inputs[k]) for k in
          ("aW2", "qW1", "qW2", "qW3")}
    pW64 = np.asarray(inputs["pW"], np.float32) * 64.0
    Wb["pW12"] = np.ascontiguousarray(pW64[0:512] * 16.0).astype(
        ml_dtypes.bfloat16)
    Wb["pW3"] = np.ascontiguousarray(pW64[512:2560]).astype(
        ml_dtypes.float8_e4m3fn)
    Wb["aW3"] = np.ascontiguousarray(
        (np.asarray(inputs["aW3"], np.float32) * 16.0).astype(
            ml_dtypes.float8_e4m3fn))
    b = {k: f32(inputs[k]) for k in
         ("b1", "b2", "b3", "ab1", "ab2", "ab3", "pb", "qb1", "qb2", "qb3",
          "mb1", "mb2")}
    b["ab3"] = b["ab3"] * 16.0  # matches the x16 fp8 aW3 (a3p holds 16*a3)
    ident = np.eye(128, dtype=np.float32)
    rows2 = _pack_rows(b)

    in_maps = []
    for c in range(NCORES):
        fs = feat[c * NLOC : (c + 1) * NLOC]
        feat_nm = np.ascontiguousarray(
            fs.reshape(T, 128, IN).transpose(1, 0, 2).reshape(128, T * IN))
        featT = np.ascontiguousarray(fs.T)
        at_idx, at_val, degl = _edge_tables(edge_src, edge_dst, c)
        pW12_lc = np.ascontiguousarray(Wb["pW12"][:, c * G * K : (c + 1) * G * K])
        pW3_lc = np.ascontiguousarray(Wb["pW3"][:, c * G * K : (c + 1) * G * K])
        pb_lc = np.ascontiguousarray(b["pb"][c * G * K : (c + 1) * G * K])
        degr = np.zeros((65, 1024), np.float32)
        for g in range(G):
            p, off = _GSLOT[g]
            degr[p, off : off + NPG] = degl[g * NPG : (g + 1) * NPG]
        in_maps.append({
            "featT": tf32_round(featT),
            "feat_nm": tf32_round(feat_nm),
            "at_idx": at_idx, "at_val": at_val,
            "bcol": _pack_bcol(b, pb_lc), "rows2": rows2,
            "ident": ident, "identr": ident,
            "degr": degr,
            "W1": W["W1"], "W2": W["W2"], "W3": W["W3"],
            "aW1": W["aW1"], "aW2": Wb["aW2"], "aW3": Wb["aW3"],
            "pW12": pW12_lc, "pW3": pW3_lc,
            "qW1": Wb["qW1"], "qW2": Wb["qW2"], "qW3": Wb["qW3"],
            "mW1": W["mW1"], "mW2": W["mW2"],
        })
    return in_maps


def kernel(**inputs):
    if "nc" not in _CACHE:
        _CACHE["nc"] = build_module()
    nc = _CACHE["nc"]
    in_maps = prepare_in_maps(inputs)
    res = run_bass_kernel_spmd(nc, in_maps, core_ids=list(range(NCORES)),
                               trace=TRACE)
    _CACHE["last_res"] = res
    out = np.zeros((B, 10), np.float32)
    for c in range(NCORES):
        out[c * G : (c + 1) * G, :] = np.asarray(res.results[c]["yp"]).T
    return out
